# revision 1
# baseline (speedup 1.0000x reference)
"""Multi-head self-attention (B=2, N=4096, D=768, H=12, dh=64) on 8 trn2 NeuronCores.

Sharding: core c handles batch b=c//4 and heads 3*(c%4)..3*(c%4)+2 (head-parallel
attention), then an AllToAll redistributes head-outputs so each core projects its
own token quarter with the full w_out (token-parallel output projection).

Per-core pipeline (all matmuls fp32r):
  1. qkv projection from host-transposed xT, producing qT/kT (d-on-partition,
     head-pair packed for PE row-tiling) and v (token-on-partition, via PE
     transpose of vT), with k pre-scaled by softmax_scale/8.
  2. flash-style attention per 512-query chunk: row-tiled QK^T -> exp (split
     between ACT table exp and a custom 2-op DVE polynomial exp) -> PV
     accumulation with an appended ones-column producing the softmax
     denominator -> normalize.
  3. AllToAll over each batch's 4 cores + output projection + bias.
"""
import sys

sys.path.insert(0, "/opt/trn_rl_repo")

import numpy as np

import concourse.bass as bass
import concourse.mybir as mybir
import concourse.tile as tile
import concourse.bacc as bacc
from concourse.masks import make_identity

N_CORES = 8
B, N, D, H, DH = 2, 4096, 768, 12, 64
HPC = 3            # heads per core
NQ = N // 4        # tokens per core quarter (1024)
SCALE = D ** -0.5
F32 = mybir.dt.float32
F32R = mybir.dt.float32r
AF = mybir.ActivationFunctionType
BF16 = mybir.dt.bfloat16
IC = 512           # query chunk (i-chunk) size
NIC = N // IC      # 8 i-chunks
NTAU = N // IC     # token chunks for projection (512 wide)
VW = 195           # v_sb row stride: [v0|1|v1|1|v2|1] = 3*65
DVE_EXP_MOD = 7    # 2 of every 7 exp batches go to the DVE poly-exp


# ---------------------------------------------------------------- custom DVE exp
def _register_exp_ops():
    """exp(8u) as two DVE ops: EXP_P4_ANT = taylor4(u); EXP_SQ8_ANT = x^8."""
    import concourse.dve_ops as dve_ops
    from concourse.dve_ops import DveOp, OPS, CUSTOM_DVE_SPECS, _SUB_OPCODE_FOR_NAME
    from concourse.dve_spec import Spec, Src0, C0, C1, C2, One, sq, lower
    from concourse.dve_uop import DveOpSpec

    if "EXP_P4_ANT" in _SUB_OPCODE_FOR_NAME:
        return dve_ops.EXP_P4_ANT, dve_ops.EXP_SQ8_ANT

    u = Src0
    p4 = ((((u * C0) + C1) * u + C2) * u + One) * u + One  # c0=1/24 c1=1/6 c2=1/2
    spec_p4 = Spec(
        body=p4,
        reference=lambda in0, in1, s0, s1, imm2: (
            (((in0 * s0 + s1) * in0 + imm2) * in0 + 1.0) * in0 + 1.0
        ),
    )
    spec_sq8 = Spec(
        body=sq(sq(sq(Src0))),
        reference=lambda in0, in1, s0, s1, imm2: in0 ** 8,
    )

    def _mk(name, spec):
        opcode = max(_SUB_OPCODE_FOR_NAME.values()) + 1
        _SUB_OPCODE_FOR_NAME[name] = opcode
        shas = {}
        for ver in ("v3", "v4"):
            s = DveOpSpec(
                name=name, opcode=opcode, uops=lower(spec, ver=ver), rd1_en=False
            )
            shas[ver] = s.sha(ver)
        op = DveOp(name, spec, subdim=False, uops_sha=shas)
        OPS.append(op)
        CUSTOM_DVE_SPECS[name] = spec
        setattr(dve_ops, name, op)
        return op

    p4_op = _mk("EXP_P4_ANT", spec_p4)
    sq8_op = _mk("EXP_SQ8_ANT", spec_sq8)
    return p4_op, sq8_op


# ---------------------------------------------------------------- program build
_PROG_CACHE = {}


def build_program(use_dve_exp=True):
    key = ("prog", use_dve_exp)
    if key in _PROG_CACHE:
        return _PROG_CACHE[key]
    p4_op, sq8_op = _register_exp_ops()

    nc = bacc.Bacc("TRN2", target_bir_lowering=False, debug=False, num_devices=N_CORES)

    xT = nc.dram_tensor("xT", [D, N], F32, kind="ExternalInput").ap()
    wqkv = nc.dram_tensor("wqkv", [D, 768], F32, kind="ExternalInput").ap()
    bqkv = nc.dram_tensor("bqkv", [6, 128], F32, kind="ExternalInput").ap()
    wout = nc.dram_tensor("wout", [D, 192], F32, kind="ExternalInput").ap()
    bout = nc.dram_tensor("bout", [2, 128], F32, kind="ExternalInput").ap()
    y = nc.dram_tensor("y", [HPC * DH, N], F32, kind="ExternalOutput").ap()

    xT_r = xT.bitcast(F32R)
    wqkv_r = wqkv.bitcast(F32R)
    wout_r = wout.bitcast(F32R)

    with tile.TileContext(nc, trace_sim=False) as tc:
        with (
            tc.tile_pool(name="consts", bufs=1) as consts,
            tc.tile_pool(name="persist", bufs=1) as persist,
            tc.tile_pool(name="otp", bufs=3) as otp,
            tc.tile_pool(name="spsum", bufs=2, space="PSUM") as spsum,
            tc.tile_pool(name="opsum", bufs=1, space="PSUM") as opsum,
            tc.tile_pool(name="dram", bufs=1, space="DRAM") as dram,
        ):
            # ---------------- constants
            ident_f = consts.tile([128, 128], F32)
            make_identity(nc, ident_f[:])
            ident = consts.tile([128, 128], F32R)
            nc.scalar.copy(ident[:], ident_f[:])

            ones_f = consts.tile([128, 64], F32)
            nc.vector.memset(ones_f[:], 1.0)
            ones1 = consts.tile([1, 64], F32R)  # K=1 stationary for r broadcast
            nc.scalar.copy(ones1[:], ones_f[0:1, :])

            wq_sb = consts.tile([128, 6 * 768], F32R)  # 6 f-chunks of [128,768]
            for fc in range(6):
                nc.sync.dma_start(
                    out=wq_sb[:, fc * 768 : (fc + 1) * 768],
                    in_=wqkv_r[fc * 128 : (fc + 1) * 128, :],
                )
            bq_sb = consts.tile([128, 6], F32)  # per-m-chunk bias columns
            for m in range(6):
                nc.sync.dma_start(
                    out=bq_sb[:, m : m + 1],
                    in_=bqkv[m : m + 1, :].rearrange("a p -> p a"),
                )

            # ---------------- persistent activations
            qT01 = persist.tile([128, N], F32R)  # rows 0:64 head0 qT, 64:128 head1
            qT2 = persist.tile([128, N], F32R)   # head2 qT duplicated in both halves
            kT01 = persist.tile([128, N], F32R)  # scaled kT, heads 0/1
            kT2 = persist.tile([128, N], F32R)   # scaled kT head2, duplicated
            v_sb = persist.tile([128, 32 * VW], F32R)  # [tok128][v0|1|v1|1|v2|1]/j-tile

            # ones columns of v_sb (positions 64,129,194 of each 195 block)
            ones32 = consts.tile([128, 32], F32)
            nc.vector.memset(ones32[:], 1.0)
            v_view = v_sb[:].rearrange("p (t c) -> p t c", c=VW)
            for col in (64, 129, 194):
                nc.scalar.copy(
                    v_view[:, :, col : col + 1],
                    ones32[:].rearrange("p (a b) -> p a b", b=1),
                )

            # AllGather eighths: in [192, 512] -> out [768, 512] (rank-major rows)
            oT_q = [dram.tile([HPC * DH, IC], BF16, name=f"oT_q{i}") for i in range(8)]
            ag_q = [dram.tile([D, IC], BF16, name=f"ag_q{i}") for i in range(8)]

            with tc.tile_pool(name="work", bufs=2) as work, \
                 tc.tile_pool(name="ptp", bufs=3) as ptp:
                # ---------------- phase 1: qkv projection
                qdst = [qT01, qT2, kT01, kT2]
                for tau in range(NTAU):
                    ts = slice(tau * IC, (tau + 1) * IC)
                    xts = []
                    for fc in range(6):
                        xt = work.tile([128, IC], F32R, tag="xt", bufs=7)
                        nc.sync.dma_start(
                            out=xt[:], in_=xT_r[fc * 128 : (fc + 1) * 128, ts]
                        )
                        xts.append(xt)
                    vt01 = work.tile([128, IC], F32R, tag="vt01")
                    vt2 = work.tile([128, IC], F32R, tag="vt2")
                    for m in range(6):
                        pp = spsum.tile([128, 1024], F32, tag="sp", bufs=3)
                        for fc in range(6):
                            nc.tensor.matmul(
                                pp[:, 0:IC],
                                wq_sb[:, fc * 768 + m * 128 : fc * 768 + (m + 1) * 128],
                                xts[fc][:],
                                start=(fc == 0),
                                stop=(fc == 5),
                            )
                        bias = bq_sb[:, m : m + 1]
                        if m < 4:
                            nc.scalar.activation(
                                qdst[m][:, ts], pp[:, 0:IC], AF.Identity, bias=bias
                            )
                        elif m == 4:
                            nc.scalar.activation(
                                vt01[:], pp[:, 0:IC], AF.Identity, bias=bias
                            )
                        else:
                            nc.scalar.activation(
                                vt2[:], pp[:, 0:IC], AF.Identity, bias=bias
                            )
                    # transpose v into token-major layout
                    for t in range(4):
                        jt = 4 * tau + t
                        base = jt * VW
                        pv = spsum.tile([128, 1024], F32R, tag="sp", bufs=3)
                        nc.tensor.transpose(
                            pv[:, 0:128], vt01[:, t * 128 : (t + 1) * 128], ident[:]
                        )
                        nc.scalar.copy(
                            v_sb[:, base : base + 130].rearrange(
                                "p (a b) -> p a b", b=65
                            )[:, :, 0:64],
                            pv[:, 0:128].rearrange("p (a b) -> p a b", b=64),
                        )
                        pv2 = spsum.tile([128, 1024], F32R, tag="sp", bufs=3)
                        nc.tensor.transpose(
                            pv2[:, 0:64],
                            vt2[0:64, t * 128 : (t + 1) * 128],
                            ident[0:64, 0:64],
                        )
                        nc.scalar.copy(v_sb[:, base + 130 : base + 194], pv2[:, 0:64])

                # ---------------- phase 2: attention
                exp_batch_idx = [0]

                def exp_batch(sp):
                    """exp(8u) on a [128,1024] psum batch -> fp32r SBUF tile."""
                    i = exp_batch_idx[0]
                    exp_batch_idx[0] += 1
                    pt = ptp.tile([128, 1024], F32R, tag="pt", bufs=5)
                    if use_dve_exp and ((2 * i) % DVE_EXP_MOD) < 2:
                        tmp = ptp.tile([128, 1024], F32, tag="exptmp", bufs=3)
                        nc.vector._custom_dve(
                            p4_op, out=tmp[:], in0=sp[:],
                            s0=1.0 / 24, s1=1.0 / 6, imm2=0.5,
                        )
                        nc.vector._custom_dve(sq8_op, out=pt[:], in0=tmp[:])
                    else:
                        nc.scalar.activation(pt[:], sp[:], AF.Exp, scale=8.0)
                    return pt

                def norm_and_store(po, ic, h):
                    """normalize [65,512] psum (row 64 = l) and DMA to oT_dram."""
                    rr = otp.tile([1, IC], F32R, tag="rr")
                    with nc.allow_low_precision(reason="fp32r is bit-identical fp32"):
                        nc.vector.reciprocal(rr[:], po[64:65, :])
                    pr = spsum.tile([128, 1024], F32, tag="sp", bufs=3)
                    nc.tensor.matmul(
                        pr[0:64, 0:IC], ones1[:], rr[:], start=True, stop=True
                    )
                    rb = otp.tile([64, IC], F32, tag="rb")
                    nc.scalar.copy(rb[:], pr[0:64, 0:IC])
                    ot = otp.tile([64, IC], BF16, tag="ot")
                    nc.vector.tensor_tensor(
                        out=ot[:], in0=po[0:64, :], in1=rb[:],
                        op=mybir.AluOpType.mult,
                    )
                    nc.sync.dma_start(
                        out=oT_q[ic][64 * h : 64 * h + 64, :], in_=ot[:]
                    )

                for ic in range(NIC):
                    isl = slice(ic * IC, (ic + 1) * IC)
                    # --- heads 0,1 (row-tiled pair over the same key tile)
                    po0 = opsum.tile([128, IC], F32, tag="po0", bufs=1)
                    po1 = opsum.tile([128, IC], F32, tag="po1", bufs=1)
                    for jc in range(32):
                        sp = spsum.tile([128, 1024], F32, tag="sp", bufs=3)
                        nc.tensor.matmul(
                            sp[:, 0:512],
                            kT01[0:64, jc * 128 : (jc + 1) * 128],
                            qT01[0:64, isl],
                            start=True, stop=True, tile_position=(0, 0),
                        )
                        nc.tensor.matmul(
                            sp[:, 512:1024],
                            kT01[64:128, jc * 128 : (jc + 1) * 128],
                            qT01[64:128, isl],
                            start=True, stop=True, tile_position=(64, 0),
                        )
                        pt = exp_batch(sp)
                        vbase = jc * VW
                        nc.tensor.matmul(
                            po0[0:65, :], v_sb[:, vbase : vbase + 65], pt[:, 0:512],
                            start=(jc == 0), stop=(jc == 31),
                        )
                        nc.tensor.matmul(
                            po1[0:65, :], v_sb[:, vbase + 65 : vbase + 130],
                            pt[:, 512:1024],
                            start=(jc == 0), stop=(jc == 31),
                        )
                    norm_and_store(po0, ic, 0)
                    norm_and_store(po1, ic, 1)
                    # --- head 2 (row-tiled pair over adjacent key tiles)
                    po2 = opsum.tile([128, IC], F32, tag="po0", bufs=1)
                    for t in range(16):
                        sp = spsum.tile([128, 1024], F32, tag="sp", bufs=3)
                        nc.tensor.matmul(
                            sp[:, 0:512],
                            kT2[0:64, (2 * t) * 128 : (2 * t + 1) * 128],
                            qT2[0:64, isl],
                            start=True, stop=True, tile_position=(0, 0),
                        )
                        nc.tensor.matmul(
                            sp[:, 512:1024],
                            kT2[64:128, (2 * t + 1) * 128 : (2 * t + 2) * 128],
                            qT2[64:128, isl],
                            start=True, stop=True, tile_position=(64, 0),
                        )
                        pt = exp_batch(sp)
                        for s in range(2):
                            jc = 2 * t + s
                            vbase = jc * VW
                            nc.tensor.matmul(
                                po2[0:65, :],
                                v_sb[:, vbase + 130 : vbase + 195],
                                pt[:, s * 512 : (s + 1) * 512],
                                start=(jc == 0), stop=(jc == 31),
                            )
                    norm_and_store(po2, ic, 2)
                    nc.gpsimd.collective_compute(
                        "AllGather",
                        mybir.AluOpType.bypass,
                        replica_groups=[[0, 1, 2, 3], [4, 5, 6, 7]],
                        ins=[oT_q[ic][:]],
                        outs=[ag_q[ic][:]],
                    )

            # ---------------- phase 3: output projection (column-parallel)
            with tc.tile_pool(name="ph3", bufs=1) as ph3:
                wo_sb = ph3.tile([128, 6 * 192], BF16)  # w_out col-slice, 6 d-chunks
                for dc in range(6):
                    nc.gpsimd.dma_start(
                        out=wo_sb[:, dc * 192 : (dc + 1) * 192],
                        in_=wout[dc * 128 : (dc + 1) * 128, :],
                    )
                bo_sb = ph3.tile([128, 2], F32)
                for m in range(2):
                    nc.sync.dma_start(
                        out=bo_sb[:, m : m + 1],
                        in_=bout[m : m + 1, :].rearrange("a p -> p a"),
                    )
                for qtr in range(8):
                    agr = ag_q[qtr][:]
                    for icl in range(1):
                        ogs = []
                        for dc in range(6):
                            og = ph3.tile([128, IC], BF16, tag="og", bufs=8)
                            nc.sync.dma_start(
                                out=og[:], in_=agr[dc * 128 : (dc + 1) * 128, :]
                            )
                            ogs.append(og)
                        for ec, (elo, ew) in enumerate(((0, 128), (128, 64))):
                            py = spsum.tile([128, 1024], F32, tag="sp", bufs=3)
                            for dc in range(6):
                                nc.tensor.matmul(
                                    py[0:ew, 0:IC],
                                    wo_sb[:, dc * 192 + elo : dc * 192 + elo + ew],
                                    ogs[dc][:],
                                    start=(dc == 0), stop=(dc == 5),
                                )
                            ysb = ph3.tile([128, IC], F32, tag="ysb", bufs=3)
                            nc.scalar.activation(
                                ysb[0:ew, :], py[0:ew, 0:IC], AF.Identity,
                                bias=bo_sb[0:ew, ec : ec + 1],
                            )
                            nc.sync.dma_start(
                                out=y[elo : elo + ew,
                                      qtr * IC : (qtr + 1) * IC],
                                in_=ysb[0:ew, :],
                            )

    nc.compile()
    _PROG_CACHE[key] = nc
    return nc


# ---------------------------------------------------------------- host wrapper
def make_in_maps(x, w_qkv, b_qkv, w_out, b_out):
    """Build the 8 per-core input dicts from full inputs."""
    in_maps = []
    xTb = [np.ascontiguousarray(x[b].T) for b in range(B)]  # [768, 4096]
    kscale = np.float32(SCALE / 8.0)
    for c in range(N_CORES):
        b = c // 4
        hs = HPC * (c % 4)

        def sect(kind, h):  # q=0,k=1,v=2
            lo = kind * (H * DH) + h * DH
            return w_qkv[:, lo : lo + DH], b_qkv[lo : lo + DH]

        q0, bq0 = sect(0, hs); q1, bq1 = sect(0, hs + 1); q2, bq2 = sect(0, hs + 2)
        k0, bk0 = sect(1, hs); k1, bk1 = sect(1, hs + 1); k2, bk2 = sect(1, hs + 2)
        v0, bv0 = sect(2, hs); v1, bv1 = sect(2, hs + 1); v2, bv2 = sect(2, hs + 2)
        z = np.zeros_like(q2); bz = np.zeros_like(bq2)
        # m-chunks: [q0|q1], [q2|q2], [k0|k1]*s, [k2|k2]*s, [v0|v1], [v2|0]
        cols = np.concatenate(
            [q0, q1, q2, q2, k0 * kscale, k1 * kscale, k2 * kscale, k2 * kscale,
             v0, v1, v2, z], axis=1).astype(np.float32)
        bias = np.concatenate(
            [bq0, bq1, bq2, bq2, bk0 * kscale, bk1 * kscale, bk2 * kscale,
             bk2 * kscale, bv0, bv1, bv2, bz]).astype(np.float32)
        q = c % 4
        bo = np.zeros((2, 128), np.float32)
        bo[0, :] = b_out[192 * q : 192 * q + 128]
        bo[1, :64] = b_out[192 * q + 128 : 192 * q + 192]
        in_maps.append({
            "xT": xTb[b],
            "wqkv": np.ascontiguousarray(cols),
            "bqkv": np.ascontiguousarray(bias.reshape(6, 128)),
            "wout": np.ascontiguousarray(
                w_out[:, 192 * q : 192 * (q + 1)].astype(np.float32)),
            "bout": bo,
        })
    return in_maps


def assemble_output(results):
    out = np.empty((B, N, D), dtype=np.float32)
    for c in range(N_CORES):
        b = c // 4
        q = c % 4
        out[b, :, 192 * q : 192 * (q + 1)] = results[c]["y"].T
    return out


def kernel(x, w_qkv, b_qkv, w_out, b_out):
    from concourse.bass_utils import run_bass_kernel_spmd

    x = np.asarray(x, dtype=np.float32)
    nc = build_program()
    in_maps = make_in_maps(
        x, np.asarray(w_qkv, np.float32), np.asarray(b_qkv, np.float32),
        np.asarray(w_out, np.float32), np.asarray(b_out, np.float32))
    res = run_bass_kernel_spmd(nc, in_maps, core_ids=list(range(N_CORES)))
    return assemble_output(res.results)



# revision 19
# speedup vs baseline: 1.1328x; 1.1328x over previous
"""Multi-head self-attention (B=2, N=4096, D=768, H=12, dh=64) on 8 trn2 NeuronCores.

Sharding: core c handles batch b=c//4 and heads 3*(c%4)..3*(c%4)+2 (head-parallel
attention), then an AllToAll redistributes head-outputs so each core projects its
own token quarter with the full w_out (token-parallel output projection).

Per-core pipeline (all matmuls fp32r):
  1. qkv projection from host-transposed xT, producing qT/kT (d-on-partition,
     head-pair packed for PE row-tiling) and v (token-on-partition, via PE
     transpose of vT), with k pre-scaled by softmax_scale/8.
  2. flash-style attention per 512-query chunk: row-tiled QK^T -> exp (split
     between ACT table exp and a custom 1-op DVE polynomial exp16) -> PV
     accumulation with an appended ones-column producing the softmax
     denominator -> normalize (fast DVE reciprocal + K=1 matmul broadcast
     into po rows 64:128).
  3. AllToAll over each batch's 4 cores + output projection + bias.
"""
import sys

sys.path.insert(0, "/opt/trn_rl_repo")

import numpy as np

import concourse.bass as bass
import concourse.mybir as mybir
import concourse.tile as tile
import concourse.bacc as bacc
from concourse.masks import make_identity

N_CORES = 8
B, N, D, H, DH = 2, 4096, 768, 12, 64
HPC = 3            # heads per core
NQ = N // 4        # tokens per core quarter (1024)
SCALE = D ** -0.5
F32 = mybir.dt.float32
F32R = mybir.dt.float32r
AF = mybir.ActivationFunctionType
BF16 = mybir.dt.bfloat16
IC = 512           # query chunk (i-chunk) size
NIC = N // IC      # 8 i-chunks
NTAU = N // IC     # token chunks for projection (512 wide)
VW = 195           # v_sb row stride: [v0|1|v1|1|v2|1] = 3*65
# 7 of every 15 exp batches go to the DVE 1-op exp16 (spread interleave)
DVE_EXP_SLOTS = frozenset((0, 2, 4, 6, 8, 10, 12))


# ---------------------------------------------------------------- custom DVE exp
def _register_exp_ops():
    """exp(16u) in ONE DVE op: (((u+1)^2 + 1) * 0.5)^16 = taylor2(u)^16.

    taylor2(u) = 1 + u + u^2/2 via ((u+1)^2 + 1)/2; 8 ALU stages exactly:
    add, sq, add, mul, sq, sq, sq, sq.  rel err ~ logit^3/1536 (<6e-3 at
    |logit|<=2), negligible after softmax averaging.
    """
    import concourse.dve_ops as dve_ops
    from concourse.dve_ops import DveOp, OPS, CUSTOM_DVE_SPECS, _SUB_OPCODE_FOR_NAME
    from concourse.dve_spec import Spec, Src0, C0, One, sq, lower
    from concourse.dve_uop import DveOpSpec

    if "EXP16_ANT" in _SUB_OPCODE_FOR_NAME:
        return dve_ops.EXP16_ANT

    u = Src0
    spec_e16 = Spec(
        body=sq(sq(sq(sq((sq(u + One) + One) * C0)))),
        reference=lambda in0, in1, s0, s1, imm2: (
            (((in0 + 1.0) ** 2 + 1.0) * s0) ** 16
        ),
    )

    def _mk(name, spec):
        opcode = max(_SUB_OPCODE_FOR_NAME.values()) + 1
        _SUB_OPCODE_FOR_NAME[name] = opcode
        shas = {}
        for ver in ("v3", "v4"):
            s = DveOpSpec(
                name=name, opcode=opcode, uops=lower(spec, ver=ver), rd1_en=False
            )
            shas[ver] = s.sha(ver)
        op = DveOp(name, spec, subdim=False, uops_sha=shas)
        OPS.append(op)
        CUSTOM_DVE_SPECS[name] = spec
        setattr(dve_ops, name, op)
        return op

    return _mk("EXP16_ANT", spec_e16)


# ---------------------------------------------------------------- program build
_PROG_CACHE = {}


def build_program(use_dve_exp=True, use_fast_recip=False):
    key = ("prog", use_dve_exp, use_fast_recip)
    if key in _PROG_CACHE:
        return _PROG_CACHE[key]
    e16_op = _register_exp_ops()

    nc = bacc.Bacc("TRN2", target_bir_lowering=False, debug=False, num_devices=N_CORES)

    xT = nc.dram_tensor("xT", [D, N], F32, kind="ExternalInput").ap()
    wqkv = nc.dram_tensor("wqkv", [D, 768], F32, kind="ExternalInput").ap()
    bqkv = nc.dram_tensor("bqkv", [6, 128], F32, kind="ExternalInput").ap()
    wout = nc.dram_tensor("wout", [D, 192], F32, kind="ExternalInput").ap()
    bout = nc.dram_tensor("bout", [2, 128], F32, kind="ExternalInput").ap()
    y = nc.dram_tensor("y", [HPC * DH, N], F32, kind="ExternalOutput").ap()

    xT_r = xT.bitcast(F32R)
    wqkv_r = wqkv.bitcast(F32R)
    wout_r = wout.bitcast(F32R)

    with tile.TileContext(nc, trace_sim=False) as tc:
        with (
            tc.tile_pool(name="consts", bufs=1) as consts,
            tc.tile_pool(name="persist", bufs=1) as persist,
            tc.tile_pool(name="otp", bufs=3) as otp,
            tc.tile_pool(name="spsum", bufs=2, space="PSUM") as spsum,
            tc.tile_pool(name="opsum", bufs=1, space="PSUM") as opsum,
            tc.tile_pool(name="dram", bufs=1, space="DRAM") as dram,
            tc.tile_pool(name="ph3", bufs=1) as ph3,
        ):
            # ---------------- constants
            ident_f = consts.tile([128, 128], F32)
            make_identity(nc, ident_f[:])
            ident = consts.tile([128, 128], F32R)
            nc.scalar.copy(ident[:], ident_f[:])

            ones_f = consts.tile([128, 64], F32)
            nc.vector.memset(ones_f[:], 1.0)
            # K=1 stationary for the 1/l broadcast; row 64 so its base
            # partition matches rr's (which must match po's l row for the
            # custom-DVE reciprocal - the DVE lane is tied to the partition).
            ones1_t = consts.tile([128, 64], F32R)
            nc.scalar.copy(ones1_t[:], ones_f[:])
            ones1 = ones1_t[64:65, :]

            wq_sb = consts.tile([128, 6 * 768], F32R)  # 6 f-chunks of [128,768]
            for fc in range(6):
                nc.sync.dma_start(
                    out=wq_sb[:, fc * 768 : (fc + 1) * 768],
                    in_=wqkv_r[fc * 128 : (fc + 1) * 128, :],
                )
            bq_sb = consts.tile([128, 6], F32)  # per-m-chunk bias columns
            for m in range(6):
                nc.sync.dma_start(
                    out=bq_sb[:, m : m + 1],
                    in_=bqkv[m : m + 1, :].rearrange("a p -> p a"),
                )

            # phase-3 weights, prefetched at program start on the gpsimd queue
            wo_sb = ph3.tile([128, 6 * 192], BF16)  # w_out col-slice, 6 d-chunks
            for dc in range(6):
                nc.gpsimd.dma_start(
                    out=wo_sb[:, dc * 192 : (dc + 1) * 192],
                    in_=wout[dc * 128 : (dc + 1) * 128, :],
                )
            bo_sb = ph3.tile([128, 2], F32)
            for m in range(2):
                nc.sync.dma_start(
                    out=bo_sb[:, m : m + 1],
                    in_=bout[m : m + 1, :].rearrange("a p -> p a"),
                )

            # ---------------- persistent activations
            qT01 = persist.tile([128, N], F32R)  # rows 0:64 head0 qT, 64:128 head1
            qT2 = persist.tile([128, N], F32R)   # head2 qT duplicated in both halves
            kT01 = persist.tile([128, N], F32R)  # scaled kT, heads 0/1
            kT2 = persist.tile([128, N], F32R)   # scaled kT head2, duplicated
            v_sb = persist.tile([128, 32 * VW], F32R)  # [tok128][v0|1|v1|1|v2|1]/j-tile

            # ones columns of v_sb (positions 64,129,194 of each 195 block)
            ones32 = consts.tile([128, 32], F32)
            nc.vector.memset(ones32[:], 1.0)
            v_view = v_sb[:].rearrange("p (t c) -> p t c", c=VW)
            for col in (64, 129, 194):
                nc.scalar.copy(
                    v_view[:, :, col : col + 1],
                    ones32[:].rearrange("p (a b) -> p a b", b=1),
                )

            # AllGather eighths: in [192, 512] -> out [768, 512] (rank-major rows)
            oT_q = [dram.tile([HPC * DH, IC], BF16, name=f"oT_q{i}") for i in range(8)]
            ag_q = [dram.tile([D, IC], BF16, name=f"ag_q{i}") for i in range(8)]

            with tc.tile_pool(name="work", bufs=2) as work, \
                 tc.tile_pool(name="ptp", bufs=3) as ptp:
                # ---------------- phase 1: qkv projection
                qdst = [qT01, qT2, kT01, kT2]
                for tau in range(NTAU):
                    ts = slice(tau * IC, (tau + 1) * IC)
                    xts = []
                    for fc in range(6):
                        xt = work.tile([128, IC], F32R, tag="xt", bufs=7)
                        nc.sync.dma_start(
                            out=xt[:], in_=xT_r[fc * 128 : (fc + 1) * 128, ts]
                        )
                        xts.append(xt)
                    vt01 = work.tile([128, IC], F32R, tag="vt01")
                    vt2 = work.tile([128, IC], F32R, tag="vt2")
                    for m in range(6):
                        pp = spsum.tile([128, 1024], F32, tag="sp", bufs=3)
                        for fc in range(6):
                            nc.tensor.matmul(
                                pp[:, 0:IC],
                                wq_sb[:, fc * 768 + m * 128 : fc * 768 + (m + 1) * 128],
                                xts[fc][:],
                                start=(fc == 0),
                                stop=(fc == 5),
                            )
                        bias = bq_sb[:, m : m + 1]
                        if m < 4:
                            nc.scalar.activation(
                                qdst[m][:, ts], pp[:, 0:IC], AF.Identity, bias=bias
                            )
                        elif m == 4:
                            nc.scalar.activation(
                                vt01[:], pp[:, 0:IC], AF.Identity, bias=bias
                            )
                        else:
                            nc.scalar.activation(
                                vt2[:], pp[:, 0:IC], AF.Identity, bias=bias
                            )
                    # transpose v into token-major layout
                    for t in range(4):
                        jt = 4 * tau + t
                        base = jt * VW
                        pv = spsum.tile([128, 1024], F32R, tag="sp", bufs=3)
                        nc.tensor.transpose(
                            pv[:, 0:128], vt01[:, t * 128 : (t + 1) * 128], ident[:]
                        )
                        nc.vector.tensor_copy(
                            v_sb[:, base : base + 64], pv[:, 0:64]
                        )
                        nc.vector.tensor_copy(
                            v_sb[:, base + 65 : base + 129], pv[:, 64:128]
                        )
                        pv2 = spsum.tile([128, 1024], F32R, tag="sp", bufs=3)
                        nc.tensor.transpose(
                            pv2[:, 0:64],
                            vt2[0:64, t * 128 : (t + 1) * 128],
                            ident[0:64, 0:64],
                        )
                        nc.vector.tensor_copy(v_sb[:, base + 130 : base + 194], pv2[:, 0:64])

                # ---------------- phase 2: attention
                exp_batch_idx = [0]

                def exp_batch(sp):
                    """exp(16u) on a [128,1024] psum batch -> fp32r SBUF tile."""
                    i = exp_batch_idx[0]
                    exp_batch_idx[0] += 1
                    pt = ptp.tile([128, 1024], F32R, tag="pt", bufs=5)
                    if use_dve_exp and (i % 15) in DVE_EXP_SLOTS:
                        nc.vector._custom_dve(e16_op, out=pt[:], in0=sp[:], s0=0.5)
                    else:
                        nc.scalar.activation(pt[:], sp[:], AF.Exp, scale=16.0)
                    return pt

                def norm_and_store(po, ic, h):
                    """normalize [65,512] psum (row 64 = l) and DMA to oT_dram.

                    1/l via the fast 1-op DVE reciprocal; broadcast it with a
                    K=1 matmul into the free rows 64:128 of po's own bank
                    (overwriting l, which is dead after the reciprocal), then
                    one DVE multiply.  No ACT work, no spare PSUM tile.
                    """
                    rr_t = otp.tile([128, IC], F32R, tag="rr")
                    rr = rr_t[64:65, :]  # base partition 64 = po's l row
                    if use_fast_recip:
                        from concourse.dve_ops import (
                            RECIP_APPROX_FAST_CONSTS as _RC,
                            RECIPROCAL_APPROX_FAST as _RF,
                        )
                        with nc.allow_low_precision(reason="fp32r rounding of 1/l"):
                            nc.vector._custom_dve(
                                _RF, out=rr, in0=po[64:65, :],
                                s0=_RC["s0"], s1=_RC["s1"], imm2=_RC["imm2"],
                            )
                    else:
                        with nc.allow_low_precision(reason="fp32r is bit-identical fp32"):
                            nc.vector.reciprocal(rr, po[64:65, :])
                    pr = spsum.tile([128, 1024], F32, tag="sp", bufs=3)
                    nc.tensor.matmul(
                        pr[0:64, 0:IC], ones1, rr,
                        start=True, stop=True,
                    )
                    rb = otp.tile([64, IC], F32, tag="rb")
                    nc.scalar.copy(rb[:], pr[0:64, 0:IC])
                    ot = otp.tile([64, IC], BF16, tag="ot")
                    nc.vector.tensor_tensor(
                        out=ot[:], in0=po[0:64, :], in1=rb[:],
                        op=mybir.AluOpType.mult,
                    )
                    nc.sync.dma_start(
                        out=oT_q[ic][64 * h : 64 * h + 64, :], in_=ot[:]
                    )

                for ic in range(NIC):
                    isl = slice(ic * IC, (ic + 1) * IC)
                    # --- heads 0,1 (row-tiled pair over the same key tile)
                    po0 = opsum.tile([128, IC], F32, tag="po0", bufs=1)
                    po1 = opsum.tile([128, IC], F32, tag="po1", bufs=1)
                    for jc in range(32):
                        sp = spsum.tile([128, 1024], F32, tag="sp", bufs=3)
                        nc.tensor.matmul(
                            sp[:, 0:512],
                            kT01[0:64, jc * 128 : (jc + 1) * 128],
                            qT01[0:64, isl],
                            start=True, stop=True, tile_position=(0, 0),
                        )
                        nc.tensor.matmul(
                            sp[:, 512:1024],
                            kT01[64:128, jc * 128 : (jc + 1) * 128],
                            qT01[64:128, isl],
                            start=True, stop=True, tile_position=(64, 0),
                        )
                        pt = exp_batch(sp)
                        vbase = jc * VW
                        nc.tensor.matmul(
                            po0[0:65, :], v_sb[:, vbase : vbase + 65], pt[:, 0:512],
                            start=(jc == 0), stop=(jc == 31),
                        )
                        nc.tensor.matmul(
                            po1[0:65, :], v_sb[:, vbase + 65 : vbase + 130],
                            pt[:, 512:1024],
                            start=(jc == 0), stop=(jc == 31),
                        )
                    norm_and_store(po0, ic, 0)
                    norm_and_store(po1, ic, 1)
                    # --- head 2 (row-tiled pair over adjacent key tiles)
                    po2 = opsum.tile([128, IC], F32, tag="po0", bufs=1)
                    for t in range(16):
                        sp = spsum.tile([128, 1024], F32, tag="sp", bufs=3)
                        nc.tensor.matmul(
                            sp[:, 0:512],
                            kT2[0:64, (2 * t) * 128 : (2 * t + 1) * 128],
                            qT2[0:64, isl],
                            start=True, stop=True, tile_position=(0, 0),
                        )
                        nc.tensor.matmul(
                            sp[:, 512:1024],
                            kT2[64:128, (2 * t + 1) * 128 : (2 * t + 2) * 128],
                            qT2[64:128, isl],
                            start=True, stop=True, tile_position=(64, 0),
                        )
                        pt = exp_batch(sp)
                        for s in range(2):
                            jc = 2 * t + s
                            vbase = jc * VW
                            nc.tensor.matmul(
                                po2[0:65, :],
                                v_sb[:, vbase + 130 : vbase + 195],
                                pt[:, s * 512 : (s + 1) * 512],
                                start=(jc == 0), stop=(jc == 31),
                            )
                    norm_and_store(po2, ic, 2)
                    nc.gpsimd.collective_compute(
                        "AllGather",
                        mybir.AluOpType.bypass,
                        replica_groups=[[0, 1, 2, 3], [4, 5, 6, 7]],
                        ins=[oT_q[ic][:]],
                        outs=[ag_q[ic][:]],
                    )

            # ---------------- phase 3: output projection (column-parallel)
            for qtr in range(8):
                agr = ag_q[qtr][:]
                ogs = []
                for dc in range(6):
                    og = ph3.tile([128, IC], BF16, tag="og", bufs=8)
                    nc.gpsimd.dma_start(
                        out=og[:], in_=agr[dc * 128 : (dc + 1) * 128, :]
                    )
                    ogs.append(og)
                for ec, (elo, ew) in enumerate(((0, 128), (128, 64))):
                    py = spsum.tile([128, 1024], F32, tag="sp", bufs=3)
                    for dc in range(6):
                        nc.tensor.matmul(
                            py[0:ew, 0:IC],
                            wo_sb[:, dc * 192 + elo : dc * 192 + elo + ew],
                            ogs[dc][:],
                            start=(dc == 0), stop=(dc == 5),
                        )
                    ysb = ph3.tile([128, IC], F32, tag="ysb", bufs=3)
                    nc.scalar.activation(
                        ysb[0:ew, :], py[0:ew, 0:IC], AF.Identity,
                        bias=bo_sb[0:ew, ec : ec + 1],
                    )
                    nc.sync.dma_start(
                        out=y[elo : elo + ew,
                              qtr * IC : (qtr + 1) * IC],
                        in_=ysb[0:ew, :],
                    )

    nc.compile()
    _PROG_CACHE[key] = nc
    return nc


# ---------------------------------------------------------------- host wrapper
def make_in_maps(x, w_qkv, b_qkv, w_out, b_out):
    """Build the 8 per-core input dicts from full inputs."""
    in_maps = []
    xTb = [np.ascontiguousarray(x[b].T) for b in range(B)]  # [768, 4096]
    kscale = np.float32(SCALE / 16.0)
    for c in range(N_CORES):
        b = c // 4
        hs = HPC * (c % 4)

        def sect(kind, h):  # q=0,k=1,v=2
            lo = kind * (H * DH) + h * DH
            return w_qkv[:, lo : lo + DH], b_qkv[lo : lo + DH]

        q0, bq0 = sect(0, hs); q1, bq1 = sect(0, hs + 1); q2, bq2 = sect(0, hs + 2)
        k0, bk0 = sect(1, hs); k1, bk1 = sect(1, hs + 1); k2, bk2 = sect(1, hs + 2)
        v0, bv0 = sect(2, hs); v1, bv1 = sect(2, hs + 1); v2, bv2 = sect(2, hs + 2)
        z = np.zeros_like(q2); bz = np.zeros_like(bq2)
        # m-chunks: [q0|q1], [q2|q2], [k0|k1]*s, [k2|k2]*s, [v0|v1], [v2|0]
        cols = np.concatenate(
            [q0, q1, q2, q2, k0 * kscale, k1 * kscale, k2 * kscale, k2 * kscale,
             v0, v1, v2, z], axis=1).astype(np.float32)
        bias = np.concatenate(
            [bq0, bq1, bq2, bq2, bk0 * kscale, bk1 * kscale, bk2 * kscale,
             bk2 * kscale, bv0, bv1, bv2, bz]).astype(np.float32)
        q = c % 4
        bo = np.zeros((2, 128), np.float32)
        bo[0, :] = b_out[192 * q : 192 * q + 128]
        bo[1, :64] = b_out[192 * q + 128 : 192 * q + 192]
        in_maps.append({
            "xT": xTb[b],
            "wqkv": np.ascontiguousarray(cols),
            "bqkv": np.ascontiguousarray(bias.reshape(6, 128)),
            "wout": np.ascontiguousarray(
                w_out[:, 192 * q : 192 * (q + 1)].astype(np.float32)),
            "bout": bo,
        })
    return in_maps


def assemble_output(results):
    out = np.empty((B, N, D), dtype=np.float32)
    for c in range(N_CORES):
        b = c // 4
        q = c % 4
        out[b, :, 192 * q : 192 * (q + 1)] = results[c]["y"].T
    return out


def kernel(x, w_qkv, b_qkv, w_out, b_out):
    from concourse.bass_utils import run_bass_kernel_spmd

    x = np.asarray(x, dtype=np.float32)
    nc = build_program()
    in_maps = make_in_maps(
        x, np.asarray(w_qkv, np.float32), np.asarray(b_qkv, np.float32),
        np.asarray(w_out, np.float32), np.asarray(b_out, np.float32))
    res = run_bass_kernel_spmd(nc, in_maps, core_ids=list(range(N_CORES)))
    return assemble_output(res.results)



# revision 21
# speedup vs baseline: 1.1549x; 1.0195x over previous
"""Multi-head self-attention (B=2, N=4096, D=768, H=12, dh=64) on 8 trn2 NeuronCores.

Sharding: core c handles batch b=c//4 and heads 3*(c%4)..3*(c%4)+2 (head-parallel
attention), then an AllToAll redistributes head-outputs so each core projects its
own token quarter with the full w_out (token-parallel output projection).

Per-core pipeline (all matmuls fp32r):
  1. qkv projection from host-transposed xT, producing qT/kT (d-on-partition,
     head-pair packed for PE row-tiling) and v (token-on-partition, via PE
     transpose of vT), with k pre-scaled by softmax_scale/8.
  2. flash-style attention per 512-query chunk: row-tiled QK^T -> exp (split
     between ACT table exp and a custom 1-op DVE polynomial exp16) -> PV
     accumulation with an appended ones-column producing the softmax
     denominator -> normalize (fast DVE reciprocal + K=1 matmul broadcast
     into po rows 64:128).
  3. AllToAll over each batch's 4 cores + output projection + bias.
"""
import sys

sys.path.insert(0, "/opt/trn_rl_repo")

import numpy as np

import concourse.bass as bass
import concourse.mybir as mybir
import concourse.tile as tile
import concourse.bacc as bacc
from concourse.masks import make_identity

N_CORES = 8
B, N, D, H, DH = 2, 4096, 768, 12, 64
HPC = 3            # heads per core
NQ = N // 4        # tokens per core quarter (1024)
SCALE = D ** -0.5
F32 = mybir.dt.float32
F32R = mybir.dt.float32r
AF = mybir.ActivationFunctionType
BF16 = mybir.dt.bfloat16
IC = 512           # query chunk (i-chunk) size
NIC = N // IC      # 8 i-chunks
NTAU = N // IC     # token chunks for projection (512 wide)
VW = 195           # v_sb row stride: [v0|1|v1|1|v2|1] = 3*65
# 7 of every 15 exp batches go to the DVE 1-op exp16 (spread interleave)
DVE_EXP_SLOTS = frozenset((0, 2, 4, 6, 8, 10, 12))


# ---------------------------------------------------------------- custom DVE exp
def _register_exp_ops():
    """exp(16u) in ONE DVE op: (((u+1)^2 + 1) * 0.5)^16 = taylor2(u)^16.

    taylor2(u) = 1 + u + u^2/2 via ((u+1)^2 + 1)/2; 8 ALU stages exactly:
    add, sq, add, mul, sq, sq, sq, sq.  rel err ~ logit^3/1536 (<6e-3 at
    |logit|<=2), negligible after softmax averaging.
    """
    import concourse.dve_ops as dve_ops
    from concourse.dve_ops import DveOp, OPS, CUSTOM_DVE_SPECS, _SUB_OPCODE_FOR_NAME
    from concourse.dve_spec import Spec, Src0, C0, One, sq, lower
    from concourse.dve_uop import DveOpSpec

    if "EXP16_ANT" in _SUB_OPCODE_FOR_NAME:
        return dve_ops.EXP16_ANT

    u = Src0
    spec_e16 = Spec(
        body=sq(sq(sq(sq((sq(u + One) + One) * C0)))),
        reference=lambda in0, in1, s0, s1, imm2: (
            (((in0 + 1.0) ** 2 + 1.0) * s0) ** 16
        ),
    )

    def _mk(name, spec):
        opcode = max(_SUB_OPCODE_FOR_NAME.values()) + 1
        _SUB_OPCODE_FOR_NAME[name] = opcode
        shas = {}
        for ver in ("v3", "v4"):
            s = DveOpSpec(
                name=name, opcode=opcode, uops=lower(spec, ver=ver), rd1_en=False
            )
            shas[ver] = s.sha(ver)
        op = DveOp(name, spec, subdim=False, uops_sha=shas)
        OPS.append(op)
        CUSTOM_DVE_SPECS[name] = spec
        setattr(dve_ops, name, op)
        return op

    return _mk("EXP16_ANT", spec_e16)


# ---------------------------------------------------------------- program build
_PROG_CACHE = {}


def build_program(use_dve_exp=True, use_fast_recip=False):
    key = ("prog", use_dve_exp, use_fast_recip)
    if key in _PROG_CACHE:
        return _PROG_CACHE[key]
    e16_op = _register_exp_ops()

    nc = bacc.Bacc("TRN2", target_bir_lowering=False, debug=False, num_devices=N_CORES)

    xT = nc.dram_tensor("xT", [D, N], F32, kind="ExternalInput").ap()
    wqkv = nc.dram_tensor("wqkv", [D, 768], F32, kind="ExternalInput").ap()
    bqkv = nc.dram_tensor("bqkv", [6, 128], F32, kind="ExternalInput").ap()
    wout = nc.dram_tensor("wout", [D, 192], F32, kind="ExternalInput").ap()
    bout = nc.dram_tensor("bout", [2, 128], F32, kind="ExternalInput").ap()
    y = nc.dram_tensor("y", [HPC * DH, N], F32, kind="ExternalOutput").ap()

    xT_r = xT.bitcast(F32R)
    wqkv_r = wqkv.bitcast(F32R)
    wout_r = wout.bitcast(F32R)

    with tile.TileContext(nc, trace_sim=False) as tc:
        with (
            tc.tile_pool(name="consts", bufs=1) as consts,
            tc.tile_pool(name="persist", bufs=1) as persist,
            tc.tile_pool(name="otp", bufs=3) as otp,
            tc.tile_pool(name="spsum", bufs=2, space="PSUM") as spsum,
            tc.tile_pool(name="opsum", bufs=1, space="PSUM") as opsum,
            tc.tile_pool(name="dram", bufs=1, space="DRAM") as dram,
            tc.tile_pool(name="ph3", bufs=1) as ph3,
        ):
            # ---------------- constants
            ident_f = consts.tile([128, 128], F32)
            make_identity(nc, ident_f[:])
            ident = consts.tile([128, 128], F32R)
            nc.scalar.copy(ident[:], ident_f[:])

            ones_f = consts.tile([128, 64], F32)
            nc.vector.memset(ones_f[:], 1.0)
            # K=1 stationary for the 1/l broadcast; row 64 so its base
            # partition matches rr's (which must match po's l row for the
            # custom-DVE reciprocal - the DVE lane is tied to the partition).
            ones1_t = consts.tile([128, 64], F32R)
            nc.scalar.copy(ones1_t[:], ones_f[:])
            ones1 = ones1_t[64:65, :]

            wq_sb = consts.tile([128, 6 * 768], F32R)  # 6 f-chunks of [128,768]
            for fc in range(6):
                nc.sync.dma_start(
                    out=wq_sb[:, fc * 768 : (fc + 1) * 768],
                    in_=wqkv_r[fc * 128 : (fc + 1) * 128, :],
                )
            bq_sb = consts.tile([128, 6], F32)  # per-m-chunk bias columns
            for m in range(6):
                nc.sync.dma_start(
                    out=bq_sb[:, m : m + 1],
                    in_=bqkv[m : m + 1, :].rearrange("a p -> p a"),
                )

            # phase-3 weights, prefetched at program start on the gpsimd queue
            wo_sb = ph3.tile([128, 6 * 192], BF16)  # w_out col-slice, 6 d-chunks
            for dc in range(6):
                nc.gpsimd.dma_start(
                    out=wo_sb[:, dc * 192 : (dc + 1) * 192],
                    in_=wout[dc * 128 : (dc + 1) * 128, :],
                )
            bo_sb = ph3.tile([128, 2], F32)
            for m in range(2):
                nc.sync.dma_start(
                    out=bo_sb[:, m : m + 1],
                    in_=bout[m : m + 1, :].rearrange("a p -> p a"),
                )

            # ---------------- persistent activations
            qT01 = persist.tile([128, N], F32R)  # rows 0:64 head0 qT, 64:128 head1
            qT2 = persist.tile([128, N], F32R)   # head2 qT duplicated in both halves
            kT01 = persist.tile([128, N], F32R)  # scaled kT, heads 0/1
            kT2 = persist.tile([128, N], F32R)   # scaled kT head2, duplicated
            v_sb = persist.tile([128, 32 * VW], F32R)  # [tok128][v0|1|v1|1|v2|1]/j-tile

            # ones columns of v_sb (positions 64,129,194 of each 195 block)
            ones32 = consts.tile([128, 32], F32)
            nc.vector.memset(ones32[:], 1.0)
            v_view = v_sb[:].rearrange("p (t c) -> p t c", c=VW)
            for col in (64, 129, 194):
                nc.scalar.copy(
                    v_view[:, :, col : col + 1],
                    ones32[:].rearrange("p (a b) -> p a b", b=1),
                )

            # AllGather eighths: in [192, 512] -> out [768, 512] (rank-major rows)
            oT_q = [dram.tile([HPC * DH, IC], BF16, name=f"oT_q{i}") for i in range(8)]
            ag_q = [dram.tile([D, IC], BF16, name=f"ag_q{i}") for i in range(8)]

            with tc.tile_pool(name="work", bufs=2) as work, \
                 tc.tile_pool(name="ptp", bufs=3) as ptp:
                # ---------------- phase 1: qkv projection
                qdst = [qT01, qT2, kT01, kT2]
                for tau in range(NTAU):
                    ts = slice(tau * IC, (tau + 1) * IC)
                    xts = []
                    for fc in range(6):
                        xt = work.tile([128, IC], F32R, tag="xt", bufs=7)
                        nc.sync.dma_start(
                            out=xt[:], in_=xT_r[fc * 128 : (fc + 1) * 128, ts]
                        )
                        xts.append(xt)
                    vt01 = work.tile([128, IC], F32R, tag="vt01")
                    vt2 = work.tile([128, IC], F32R, tag="vt2")
                    for m in range(6):
                        pp = spsum.tile([128, 1024], F32, tag="sp", bufs=3)
                        for fc in range(6):
                            nc.tensor.matmul(
                                pp[:, 0:IC],
                                wq_sb[:, fc * 768 + m * 128 : fc * 768 + (m + 1) * 128],
                                xts[fc][:],
                                start=(fc == 0),
                                stop=(fc == 5),
                            )
                        bias = bq_sb[:, m : m + 1]
                        if m < 4:
                            nc.scalar.activation(
                                qdst[m][:, ts], pp[:, 0:IC], AF.Identity, bias=bias
                            )
                        elif m == 4:
                            nc.scalar.activation(
                                vt01[:], pp[:, 0:IC], AF.Identity, bias=bias
                            )
                        else:
                            nc.scalar.activation(
                                vt2[:], pp[:, 0:IC], AF.Identity, bias=bias
                            )
                    # transpose v into token-major layout
                    for t in range(4):
                        jt = 4 * tau + t
                        base = jt * VW
                        pv = spsum.tile([128, 1024], F32R, tag="sp", bufs=3)
                        nc.tensor.transpose(
                            pv[:, 0:128], vt01[:, t * 128 : (t + 1) * 128], ident[:]
                        )
                        nc.vector.tensor_copy(
                            v_sb[:, base : base + 64], pv[:, 0:64]
                        )
                        nc.vector.tensor_copy(
                            v_sb[:, base + 65 : base + 129], pv[:, 64:128]
                        )
                        pv2 = spsum.tile([128, 1024], F32R, tag="sp", bufs=3)
                        nc.tensor.transpose(
                            pv2[:, 0:64],
                            vt2[0:64, t * 128 : (t + 1) * 128],
                            ident[0:64, 0:64],
                        )
                        nc.vector.tensor_copy(v_sb[:, base + 130 : base + 194], pv2[:, 0:64])

                # ---------------- phase 2: attention
                exp_batch_idx = [0]

                def exp_batch(sp):
                    """exp(16u) on a [128,1024] psum batch -> fp32r SBUF tile."""
                    i = exp_batch_idx[0]
                    exp_batch_idx[0] += 1
                    pt = ptp.tile([128, 1024], F32R, tag="pt", bufs=5)
                    if use_dve_exp and (i % 15) in DVE_EXP_SLOTS:
                        nc.vector._custom_dve(e16_op, out=pt[:], in0=sp[:], s0=0.5)
                    else:
                        nc.scalar.activation(pt[:], sp[:], AF.Exp, scale=16.0)
                    return pt

                def norm_and_store(po, ic, h):
                    """normalize [65,512] psum (row 64 = l) and DMA to oT_dram.

                    1/l via the fast 1-op DVE reciprocal; broadcast it with a
                    K=1 matmul into the free rows 64:128 of po's own bank
                    (overwriting l, which is dead after the reciprocal), then
                    one DVE multiply.  No ACT work, no spare PSUM tile.
                    """
                    rr_t = otp.tile([128, IC], F32R, tag="rr")
                    rr = rr_t[64:65, :]  # base partition 64 = po's l row
                    if use_fast_recip:
                        from concourse.dve_ops import (
                            RECIP_APPROX_FAST_CONSTS as _RC,
                            RECIPROCAL_APPROX_FAST as _RF,
                        )
                        with nc.allow_low_precision(reason="fp32r rounding of 1/l"):
                            nc.vector._custom_dve(
                                _RF, out=rr, in0=po[64:65, :],
                                s0=_RC["s0"], s1=_RC["s1"], imm2=_RC["imm2"],
                            )
                    else:
                        with nc.allow_low_precision(reason="fp32r is bit-identical fp32"):
                            nc.vector.reciprocal(rr, po[64:65, :])
                    pr = spsum.tile([128, 1024], F32, tag="sp", bufs=3)
                    nc.tensor.matmul(
                        pr[0:64, 0:IC], ones1, rr,
                        start=True, stop=True,
                    )
                    rb = otp.tile([64, IC], F32, tag="rb")
                    nc.scalar.copy(rb[:], pr[0:64, 0:IC])
                    ot = otp.tile([64, IC], BF16, tag="ot")
                    nc.vector.tensor_tensor(
                        out=ot[:], in0=po[0:64, :], in1=rb[:],
                        op=mybir.AluOpType.mult,
                    )
                    nc.sync.dma_start(
                        out=oT_q[ic][64 * h : 64 * h + 64, :], in_=ot[:]
                    )

                # Software-pipelined slot stream: QK(s)+exp(s) are emitted at
                # slot s, but the PV matmuls for slot s are deferred to slot
                # s+2.  The PE engine queue is strict FIFO, so in the naive
                # order every PV stalls the queue waiting on its exp and the
                # PE idles in small gaps each slot (HAM then never reaches
                # K=8/8 and all matmuls run at half clock).  With the 2-slot
                # deferral each exp has ~2 full slot periods to finish before
                # its PV reaches the head of the PE queue.
                pending = []  # deferred PV-emission closures (depth 2)

                def push_slot(pv_fn):
                    pending.append(pv_fn)
                    if len(pending) > 2:
                        pending.pop(0)()

                for ic in range(NIC):
                    isl = slice(ic * IC, (ic + 1) * IC)
                    # --- heads 0,1 (row-tiled pair over the same key tile)
                    po0 = opsum.tile([128, IC], F32, tag="po0", bufs=1)
                    po1 = opsum.tile([128, IC], F32, tag="po1", bufs=1)
                    po2_box = [None]
                    for jc in range(32):
                        sp = spsum.tile([128, 1024], F32, tag="sp", bufs=3)
                        nc.tensor.matmul(
                            sp[:, 0:512],
                            kT01[0:64, jc * 128 : (jc + 1) * 128],
                            qT01[0:64, isl],
                            start=True, stop=True, tile_position=(0, 0),
                        )
                        nc.tensor.matmul(
                            sp[:, 512:1024],
                            kT01[64:128, jc * 128 : (jc + 1) * 128],
                            qT01[64:128, isl],
                            start=True, stop=True, tile_position=(64, 0),
                        )
                        pt = exp_batch(sp)

                        def pv_h01(pt=pt, jc=jc, po0=po0, po1=po1, ic=ic):
                            vbase = jc * VW
                            nc.tensor.matmul(
                                po0[0:65, :], v_sb[:, vbase : vbase + 65],
                                pt[:, 0:512],
                                start=(jc == 0), stop=(jc == 31),
                            )
                            nc.tensor.matmul(
                                po1[0:65, :], v_sb[:, vbase + 65 : vbase + 130],
                                pt[:, 512:1024],
                                start=(jc == 0), stop=(jc == 31),
                            )
                            if jc == 31:
                                norm_and_store(po0, ic, 0)
                                norm_and_store(po1, ic, 1)

                        push_slot(pv_h01)
                    # --- head 2 (row-tiled pair over adjacent key tiles)
                    for t in range(16):
                        sp = spsum.tile([128, 1024], F32, tag="sp", bufs=3)
                        nc.tensor.matmul(
                            sp[:, 0:512],
                            kT2[0:64, (2 * t) * 128 : (2 * t + 1) * 128],
                            qT2[0:64, isl],
                            start=True, stop=True, tile_position=(0, 0),
                        )
                        nc.tensor.matmul(
                            sp[:, 512:1024],
                            kT2[64:128, (2 * t + 1) * 128 : (2 * t + 2) * 128],
                            qT2[64:128, isl],
                            start=True, stop=True, tile_position=(64, 0),
                        )
                        pt = exp_batch(sp)

                        def pv_h2(pt=pt, t=t, ic=ic, po2_box=po2_box):
                            # allocated lazily so the buffer handoff from po0
                            # (shared tag) happens after po0's norm is emitted
                            if po2_box[0] is None:
                                po2_box[0] = opsum.tile(
                                    [128, IC], F32, tag="po0", bufs=1,
                                    name=f"po2_{ic}",
                                )
                            po2 = po2_box[0]
                            for s in range(2):
                                jc = 2 * t + s
                                vbase = jc * VW
                                nc.tensor.matmul(
                                    po2[0:65, :],
                                    v_sb[:, vbase + 130 : vbase + 195],
                                    pt[:, s * 512 : (s + 1) * 512],
                                    start=(jc == 0), stop=(jc == 31),
                                )
                            if t == 15:
                                norm_and_store(po2, ic, 2)
                                nc.gpsimd.collective_compute(
                                    "AllGather",
                                    mybir.AluOpType.bypass,
                                    replica_groups=[[0, 1, 2, 3], [4, 5, 6, 7]],
                                    ins=[oT_q[ic][:]],
                                    outs=[ag_q[ic][:]],
                                )

                        push_slot(pv_h2)
                while pending:
                    pending.pop(0)()

            # ---------------- phase 3: output projection (column-parallel)
            for qtr in range(8):
                agr = ag_q[qtr][:]
                ogs = []
                for dc in range(6):
                    og = ph3.tile([128, IC], BF16, tag="og", bufs=8)
                    nc.gpsimd.dma_start(
                        out=og[:], in_=agr[dc * 128 : (dc + 1) * 128, :]
                    )
                    ogs.append(og)
                for ec, (elo, ew) in enumerate(((0, 128), (128, 64))):
                    py = spsum.tile([128, 1024], F32, tag="sp", bufs=3)
                    for dc in range(6):
                        nc.tensor.matmul(
                            py[0:ew, 0:IC],
                            wo_sb[:, dc * 192 + elo : dc * 192 + elo + ew],
                            ogs[dc][:],
                            start=(dc == 0), stop=(dc == 5),
                        )
                    ysb = ph3.tile([128, IC], F32, tag="ysb", bufs=3)
                    nc.scalar.activation(
                        ysb[0:ew, :], py[0:ew, 0:IC], AF.Identity,
                        bias=bo_sb[0:ew, ec : ec + 1],
                    )
                    nc.sync.dma_start(
                        out=y[elo : elo + ew,
                              qtr * IC : (qtr + 1) * IC],
                        in_=ysb[0:ew, :],
                    )

    nc.compile()
    _PROG_CACHE[key] = nc
    return nc


# ---------------------------------------------------------------- host wrapper
def make_in_maps(x, w_qkv, b_qkv, w_out, b_out):
    """Build the 8 per-core input dicts from full inputs."""
    in_maps = []
    xTb = [np.ascontiguousarray(x[b].T) for b in range(B)]  # [768, 4096]
    kscale = np.float32(SCALE / 16.0)
    for c in range(N_CORES):
        b = c // 4
        hs = HPC * (c % 4)

        def sect(kind, h):  # q=0,k=1,v=2
            lo = kind * (H * DH) + h * DH
            return w_qkv[:, lo : lo + DH], b_qkv[lo : lo + DH]

        q0, bq0 = sect(0, hs); q1, bq1 = sect(0, hs + 1); q2, bq2 = sect(0, hs + 2)
        k0, bk0 = sect(1, hs); k1, bk1 = sect(1, hs + 1); k2, bk2 = sect(1, hs + 2)
        v0, bv0 = sect(2, hs); v1, bv1 = sect(2, hs + 1); v2, bv2 = sect(2, hs + 2)
        z = np.zeros_like(q2); bz = np.zeros_like(bq2)
        # m-chunks: [q0|q1], [q2|q2], [k0|k1]*s, [k2|k2]*s, [v0|v1], [v2|0]
        cols = np.concatenate(
            [q0, q1, q2, q2, k0 * kscale, k1 * kscale, k2 * kscale, k2 * kscale,
             v0, v1, v2, z], axis=1).astype(np.float32)
        bias = np.concatenate(
            [bq0, bq1, bq2, bq2, bk0 * kscale, bk1 * kscale, bk2 * kscale,
             bk2 * kscale, bv0, bv1, bv2, bz]).astype(np.float32)
        q = c % 4
        bo = np.zeros((2, 128), np.float32)
        bo[0, :] = b_out[192 * q : 192 * q + 128]
        bo[1, :64] = b_out[192 * q + 128 : 192 * q + 192]
        in_maps.append({
            "xT": xTb[b],
            "wqkv": np.ascontiguousarray(cols),
            "bqkv": np.ascontiguousarray(bias.reshape(6, 128)),
            "wout": np.ascontiguousarray(
                w_out[:, 192 * q : 192 * (q + 1)].astype(np.float32)),
            "bout": bo,
        })
    return in_maps


def assemble_output(results):
    out = np.empty((B, N, D), dtype=np.float32)
    for c in range(N_CORES):
        b = c // 4
        q = c % 4
        out[b, :, 192 * q : 192 * (q + 1)] = results[c]["y"].T
    return out


def kernel(x, w_qkv, b_qkv, w_out, b_out):
    from concourse.bass_utils import run_bass_kernel_spmd

    x = np.asarray(x, dtype=np.float32)
    nc = build_program()
    in_maps = make_in_maps(
        x, np.asarray(w_qkv, np.float32), np.asarray(b_qkv, np.float32),
        np.asarray(w_out, np.float32), np.asarray(b_out, np.float32))
    res = run_bass_kernel_spmd(nc, in_maps, core_ids=list(range(N_CORES)))
    return assemble_output(res.results)



# revision 22
# speedup vs baseline: 1.3476x; 1.1669x over previous
"""Multi-head self-attention (B=2, N=4096, D=768, H=12, dh=64) on 8 trn2 NeuronCores.

Sharding: core c handles batch b=c//4 and heads 3*(c%4)..3*(c%4)+2 (head-parallel
attention), then an AllToAll redistributes head-outputs so each core projects its
own token quarter with the full w_out (token-parallel output projection).

Per-core pipeline (all matmuls fp32r):
  1. qkv projection from host-transposed xT, producing qT/kT (d-on-partition,
     head-pair packed for PE row-tiling) and v (token-on-partition, via PE
     transpose of vT), with k pre-scaled by softmax_scale/8.
  2. flash-style attention per 512-query chunk: row-tiled QK^T -> exp (split
     between ACT table exp and a custom 1-op DVE polynomial exp16) -> PV
     accumulation with an appended ones-column producing the softmax
     denominator -> normalize (fast DVE reciprocal + K=1 matmul broadcast
     into po rows 64:128).
  3. AllToAll over each batch's 4 cores + output projection + bias.
"""
import sys

sys.path.insert(0, "/opt/trn_rl_repo")

import numpy as np

import concourse.bass as bass
import concourse.mybir as mybir
import concourse.tile as tile
import concourse.bacc as bacc
from concourse.masks import make_identity

N_CORES = 8
B, N, D, H, DH = 2, 4096, 768, 12, 64
HPC = 3            # heads per core
NQ = N // 4        # tokens per core quarter (1024)
SCALE = D ** -0.5
F32 = mybir.dt.float32
F32R = mybir.dt.float32r
AF = mybir.ActivationFunctionType
BF16 = mybir.dt.bfloat16
IC = 512           # query chunk (i-chunk) size
NIC = N // IC      # 8 i-chunks
NTAU = N // IC     # token chunks for projection (512 wide)
VW = 195           # v_sb row stride: [v0|1|v1|1|v2|1] = 3*65
# 5 of every 12 exp batches go to the DVE 1-op exp16 (spread interleave)
DVE_EXP_MOD = 12
DVE_EXP_SLOTS = frozenset((0, 2, 5, 7, 10))


# ---------------------------------------------------------------- custom DVE exp
def _register_exp_ops():
    """exp(16u) in ONE DVE op: (((u+1)^2 + 1) * 0.5)^16 = taylor2(u)^16.

    taylor2(u) = 1 + u + u^2/2 via ((u+1)^2 + 1)/2; 8 ALU stages exactly:
    add, sq, add, mul, sq, sq, sq, sq.  rel err ~ logit^3/1536 (<6e-3 at
    |logit|<=2), negligible after softmax averaging.
    """
    import concourse.dve_ops as dve_ops
    from concourse.dve_ops import DveOp, OPS, CUSTOM_DVE_SPECS, _SUB_OPCODE_FOR_NAME
    from concourse.dve_spec import Spec, Src0, C0, One, sq, lower
    from concourse.dve_uop import DveOpSpec

    if "EXP16_ANT" in _SUB_OPCODE_FOR_NAME:
        return dve_ops.EXP16_ANT

    u = Src0
    spec_e16 = Spec(
        body=sq(sq(sq(sq((sq(u + One) + One) * C0)))),
        reference=lambda in0, in1, s0, s1, imm2: (
            (((in0 + 1.0) ** 2 + 1.0) * s0) ** 16
        ),
    )

    def _mk(name, spec):
        opcode = max(_SUB_OPCODE_FOR_NAME.values()) + 1
        _SUB_OPCODE_FOR_NAME[name] = opcode
        shas = {}
        for ver in ("v3", "v4"):
            s = DveOpSpec(
                name=name, opcode=opcode, uops=lower(spec, ver=ver), rd1_en=False
            )
            shas[ver] = s.sha(ver)
        op = DveOp(name, spec, subdim=False, uops_sha=shas)
        OPS.append(op)
        CUSTOM_DVE_SPECS[name] = spec
        setattr(dve_ops, name, op)
        return op

    return _mk("EXP16_ANT", spec_e16)


# ---------------------------------------------------------------- program build
_PROG_CACHE = {}


def build_program(use_dve_exp=True, use_fast_recip=False):
    key = ("prog", use_dve_exp, use_fast_recip)
    if key in _PROG_CACHE:
        return _PROG_CACHE[key]
    e16_op = _register_exp_ops()

    nc = bacc.Bacc("TRN2", target_bir_lowering=False, debug=False, num_devices=N_CORES)

    xT = nc.dram_tensor("xT", [D, N], F32, kind="ExternalInput").ap()
    wqkv = nc.dram_tensor("wqkv", [D, 768], F32, kind="ExternalInput").ap()
    bqkv = nc.dram_tensor("bqkv", [6, 128], F32, kind="ExternalInput").ap()
    wout = nc.dram_tensor("wout", [D, 192], F32, kind="ExternalInput").ap()
    bout = nc.dram_tensor("bout", [2, 128], F32, kind="ExternalInput").ap()
    y = nc.dram_tensor("y", [HPC * DH, N], F32, kind="ExternalOutput").ap()

    xT_r = xT.bitcast(F32R)
    wqkv_r = wqkv.bitcast(F32R)
    wout_r = wout.bitcast(F32R)

    with tile.TileContext(nc, trace_sim=False) as tc:
        with (
            tc.tile_pool(name="consts", bufs=1) as consts,
            tc.tile_pool(name="persist", bufs=1) as persist,
            tc.tile_pool(name="otp", bufs=3) as otp,
            tc.tile_pool(name="spsum", bufs=2, space="PSUM") as spsum,
            tc.tile_pool(name="opsum", bufs=1, space="PSUM") as opsum,
            tc.tile_pool(name="dram", bufs=1, space="DRAM") as dram,
            tc.tile_pool(name="ph3", bufs=1) as ph3,
        ):
            # ---------------- constants
            ident_f = consts.tile([128, 128], F32)
            make_identity(nc, ident_f[:])
            ident = consts.tile([128, 128], BF16)
            nc.scalar.copy(ident[:], ident_f[:])

            ones_f = consts.tile([128, 64], F32)
            nc.vector.memset(ones_f[:], 1.0)
            # K=1 stationary for the 1/l broadcast; row 64 so its base
            # partition matches rr's (which must match po's l row for the
            # custom-DVE reciprocal - the DVE lane is tied to the partition).
            ones1_t = consts.tile([128, 64], F32R)
            nc.scalar.copy(ones1_t[:], ones_f[:])
            ones1 = ones1_t[64:65, :]

            wq_sb = consts.tile([128, 6 * 768], F32R)  # 6 f-chunks of [128,768]
            for fc in range(6):
                nc.sync.dma_start(
                    out=wq_sb[:, fc * 768 : (fc + 1) * 768],
                    in_=wqkv_r[fc * 128 : (fc + 1) * 128, :],
                )
            bq_sb = consts.tile([128, 6], F32)  # per-m-chunk bias columns
            for m in range(6):
                nc.sync.dma_start(
                    out=bq_sb[:, m : m + 1],
                    in_=bqkv[m : m + 1, :].rearrange("a p -> p a"),
                )

            # phase-3 weights, prefetched at program start on the gpsimd queue
            wo_sb = ph3.tile([128, 6 * 192], BF16)  # w_out col-slice, 6 d-chunks
            for dc in range(6):
                nc.gpsimd.dma_start(
                    out=wo_sb[:, dc * 192 : (dc + 1) * 192],
                    in_=wout[dc * 128 : (dc + 1) * 128, :],
                )
            bo_sb = ph3.tile([128, 2], F32)
            for m in range(2):
                nc.sync.dma_start(
                    out=bo_sb[:, m : m + 1],
                    in_=bout[m : m + 1, :].rearrange("a p -> p a"),
                )

            # ---------------- persistent activations
            qT01 = persist.tile([128, N], BF16)  # rows 0:64 head0 qT, 64:128 head1
            qT2 = persist.tile([128, N], BF16)   # head2 qT duplicated in both halves
            kT01 = persist.tile([128, N], BF16)  # scaled kT, heads 0/1
            kT2 = persist.tile([128, N], BF16)   # scaled kT head2, duplicated
            v_sb = persist.tile([128, 32 * VW], BF16)  # [tok128][v0|1|v1|1|v2|1]/j-tile

            # ones columns of v_sb (positions 64,129,194 of each 195 block)
            ones32 = consts.tile([128, 32], F32)
            nc.vector.memset(ones32[:], 1.0)
            v_view = v_sb[:].rearrange("p (t c) -> p t c", c=VW)
            for col in (64, 129, 194):
                nc.scalar.copy(
                    v_view[:, :, col : col + 1],
                    ones32[:].rearrange("p (a b) -> p a b", b=1),
                )

            # AllGather eighths: in [192, 512] -> out [768, 512] (rank-major rows)
            oT_q = [dram.tile([HPC * DH, IC], BF16, name=f"oT_q{i}") for i in range(8)]
            ag_q = [dram.tile([D, IC], BF16, name=f"ag_q{i}") for i in range(8)]

            with tc.tile_pool(name="work", bufs=2) as work, \
                 tc.tile_pool(name="ptp", bufs=3) as ptp:
                # ---------------- phase 1: qkv projection
                qdst = [qT01, qT2, kT01, kT2]
                for tau in range(NTAU):
                    ts = slice(tau * IC, (tau + 1) * IC)
                    xts = []
                    for fc in range(6):
                        xt = work.tile([128, IC], F32R, tag="xt", bufs=7)
                        nc.sync.dma_start(
                            out=xt[:], in_=xT_r[fc * 128 : (fc + 1) * 128, ts]
                        )
                        xts.append(xt)
                    vt01 = work.tile([128, IC], BF16, tag="vt01")
                    vt2 = work.tile([128, IC], BF16, tag="vt2")
                    for m in range(6):
                        pp = spsum.tile([128, 1024], F32, tag="sp", bufs=3)
                        for fc in range(6):
                            nc.tensor.matmul(
                                pp[:, 0:IC],
                                wq_sb[:, fc * 768 + m * 128 : fc * 768 + (m + 1) * 128],
                                xts[fc][:],
                                start=(fc == 0),
                                stop=(fc == 5),
                            )
                        bias = bq_sb[:, m : m + 1]
                        if m < 4:
                            nc.scalar.activation(
                                qdst[m][:, ts], pp[:, 0:IC], AF.Identity, bias=bias
                            )
                        elif m == 4:
                            nc.scalar.activation(
                                vt01[:], pp[:, 0:IC], AF.Identity, bias=bias
                            )
                        else:
                            nc.scalar.activation(
                                vt2[:], pp[:, 0:IC], AF.Identity, bias=bias
                            )
                    # transpose v into token-major layout
                    for t in range(4):
                        jt = 4 * tau + t
                        base = jt * VW
                        pv = spsum.tile([128, 1024], BF16, tag="sp", bufs=3)
                        nc.tensor.transpose(
                            pv[:, 0:128], vt01[:, t * 128 : (t + 1) * 128], ident[:]
                        )
                        nc.vector.tensor_copy(
                            v_sb[:, base : base + 64], pv[:, 0:64]
                        )
                        nc.vector.tensor_copy(
                            v_sb[:, base + 65 : base + 129], pv[:, 64:128]
                        )
                        pv2 = spsum.tile([128, 1024], BF16, tag="sp", bufs=3)
                        nc.tensor.transpose(
                            pv2[:, 0:64],
                            vt2[0:64, t * 128 : (t + 1) * 128],
                            ident[0:64, 0:64],
                        )
                        nc.vector.tensor_copy(v_sb[:, base + 130 : base + 194], pv2[:, 0:64])

                # ---------------- phase 2: attention
                exp_batch_idx = [0]

                def exp_batch(sp):
                    """exp(16u) on a [128,1024] psum batch -> fp32r SBUF tile."""
                    i = exp_batch_idx[0]
                    exp_batch_idx[0] += 1
                    pt = ptp.tile([128, 1024], BF16, tag="pt", bufs=5)
                    if use_dve_exp and (i % DVE_EXP_MOD) in DVE_EXP_SLOTS:
                        nc.vector._custom_dve(e16_op, out=pt[:], in0=sp[:], s0=0.5)
                    else:
                        nc.scalar.activation(pt[:], sp[:], AF.Exp, scale=16.0)
                    return pt

                def norm_and_store(po, ic, h):
                    """normalize [65,512] psum (row 64 = l) and DMA to oT_dram.

                    1/l via the fast 1-op DVE reciprocal; broadcast it with a
                    K=1 matmul into the free rows 64:128 of po's own bank
                    (overwriting l, which is dead after the reciprocal), then
                    one DVE multiply.  No ACT work, no spare PSUM tile.
                    """
                    rr_t = otp.tile([128, IC], F32R, tag="rr")
                    rr = rr_t[64:65, :]  # base partition 64 = po's l row
                    if use_fast_recip:
                        from concourse.dve_ops import (
                            RECIP_APPROX_FAST_CONSTS as _RC,
                            RECIPROCAL_APPROX_FAST as _RF,
                        )
                        with nc.allow_low_precision(reason="fp32r rounding of 1/l"):
                            nc.vector._custom_dve(
                                _RF, out=rr, in0=po[64:65, :],
                                s0=_RC["s0"], s1=_RC["s1"], imm2=_RC["imm2"],
                            )
                    else:
                        with nc.allow_low_precision(reason="fp32r is bit-identical fp32"):
                            nc.vector.reciprocal(rr, po[64:65, :])
                    pr = spsum.tile([128, 1024], F32, tag="sp", bufs=3)
                    nc.tensor.matmul(
                        pr[0:64, 0:IC], ones1, rr,
                        start=True, stop=True,
                    )
                    rb = otp.tile([64, IC], F32, tag="rb")
                    nc.scalar.copy(rb[:], pr[0:64, 0:IC])
                    ot = otp.tile([64, IC], BF16, tag="ot")
                    nc.vector.tensor_tensor(
                        out=ot[:], in0=po[0:64, :], in1=rb[:],
                        op=mybir.AluOpType.mult,
                    )
                    nc.sync.dma_start(
                        out=oT_q[ic][64 * h : 64 * h + 64, :], in_=ot[:]
                    )

                # Software-pipelined slot stream: QK(s)+exp(s) are emitted at
                # slot s, but the PV matmuls for slot s are deferred to slot
                # s+2.  The PE engine queue is strict FIFO, so in the naive
                # order every PV stalls the queue waiting on its exp and the
                # PE idles in small gaps each slot (HAM then never reaches
                # K=8/8 and all matmuls run at half clock).  With the 2-slot
                # deferral each exp has ~2 full slot periods to finish before
                # its PV reaches the head of the PE queue.
                pending = []  # deferred PV-emission closures (depth 2)

                def push_slot(pv_fn):
                    pending.append(pv_fn)
                    if len(pending) > 2:
                        pending.pop(0)()

                for ic in range(NIC):
                    isl = slice(ic * IC, (ic + 1) * IC)
                    # --- heads 0,1 (row-tiled pair over the same key tile)
                    po0 = opsum.tile([128, IC], F32, tag="po0", bufs=1)
                    po1 = opsum.tile([128, IC], F32, tag="po1", bufs=1)
                    po2_box = [None]
                    for jc in range(32):
                        sp = spsum.tile([128, 1024], F32, tag="sp", bufs=3)
                        nc.tensor.matmul(
                            sp[:, 0:512],
                            kT01[0:64, jc * 128 : (jc + 1) * 128],
                            qT01[0:64, isl],
                            start=True, stop=True, tile_position=(0, 0),
                        )
                        nc.tensor.matmul(
                            sp[:, 512:1024],
                            kT01[64:128, jc * 128 : (jc + 1) * 128],
                            qT01[64:128, isl],
                            start=True, stop=True, tile_position=(64, 0),
                        )
                        pt = exp_batch(sp)

                        def pv_h01(pt=pt, jc=jc, po0=po0, po1=po1, ic=ic):
                            vbase = jc * VW
                            nc.tensor.matmul(
                                po0[0:65, :], v_sb[:, vbase : vbase + 65],
                                pt[:, 0:512],
                                start=(jc == 0), stop=(jc == 31),
                            )
                            nc.tensor.matmul(
                                po1[0:65, :], v_sb[:, vbase + 65 : vbase + 130],
                                pt[:, 512:1024],
                                start=(jc == 0), stop=(jc == 31),
                            )
                            if jc == 31:
                                norm_and_store(po0, ic, 0)
                                norm_and_store(po1, ic, 1)

                        push_slot(pv_h01)
                    # --- head 2 (row-tiled pair over adjacent key tiles)
                    for t in range(16):
                        sp = spsum.tile([128, 1024], F32, tag="sp", bufs=3)
                        nc.tensor.matmul(
                            sp[:, 0:512],
                            kT2[0:64, (2 * t) * 128 : (2 * t + 1) * 128],
                            qT2[0:64, isl],
                            start=True, stop=True, tile_position=(0, 0),
                        )
                        nc.tensor.matmul(
                            sp[:, 512:1024],
                            kT2[64:128, (2 * t + 1) * 128 : (2 * t + 2) * 128],
                            qT2[64:128, isl],
                            start=True, stop=True, tile_position=(64, 0),
                        )
                        pt = exp_batch(sp)

                        def pv_h2(pt=pt, t=t, ic=ic, po2_box=po2_box):
                            # allocated lazily so the buffer handoff from po0
                            # (shared tag) happens after po0's norm is emitted
                            if po2_box[0] is None:
                                po2_box[0] = opsum.tile(
                                    [128, IC], F32, tag="po0", bufs=1,
                                    name=f"po2_{ic}",
                                )
                            po2 = po2_box[0]
                            for s in range(2):
                                jc = 2 * t + s
                                vbase = jc * VW
                                nc.tensor.matmul(
                                    po2[0:65, :],
                                    v_sb[:, vbase + 130 : vbase + 195],
                                    pt[:, s * 512 : (s + 1) * 512],
                                    start=(jc == 0), stop=(jc == 31),
                                )
                            if t == 15:
                                norm_and_store(po2, ic, 2)
                                nc.gpsimd.collective_compute(
                                    "AllGather",
                                    mybir.AluOpType.bypass,
                                    replica_groups=[[0, 1, 2, 3], [4, 5, 6, 7]],
                                    ins=[oT_q[ic][:]],
                                    outs=[ag_q[ic][:]],
                                )

                        push_slot(pv_h2)
                while pending:
                    pending.pop(0)()

            # ---------------- phase 3: output projection (column-parallel)
            for qtr in range(8):
                agr = ag_q[qtr][:]
                ogs = []
                for dc in range(6):
                    og = ph3.tile([128, IC], BF16, tag="og", bufs=8)
                    nc.gpsimd.dma_start(
                        out=og[:], in_=agr[dc * 128 : (dc + 1) * 128, :]
                    )
                    ogs.append(og)
                for ec, (elo, ew) in enumerate(((0, 128), (128, 64))):
                    py = spsum.tile([128, 1024], F32, tag="sp", bufs=3)
                    for dc in range(6):
                        nc.tensor.matmul(
                            py[0:ew, 0:IC],
                            wo_sb[:, dc * 192 + elo : dc * 192 + elo + ew],
                            ogs[dc][:],
                            start=(dc == 0), stop=(dc == 5),
                        )
                    ysb = ph3.tile([128, IC], F32, tag="ysb", bufs=3)
                    nc.scalar.activation(
                        ysb[0:ew, :], py[0:ew, 0:IC], AF.Identity,
                        bias=bo_sb[0:ew, ec : ec + 1],
                    )
                    nc.sync.dma_start(
                        out=y[elo : elo + ew,
                              qtr * IC : (qtr + 1) * IC],
                        in_=ysb[0:ew, :],
                    )

    nc.compile()
    _PROG_CACHE[key] = nc
    return nc


# ---------------------------------------------------------------- host wrapper
def make_in_maps(x, w_qkv, b_qkv, w_out, b_out):
    """Build the 8 per-core input dicts from full inputs."""
    in_maps = []
    xTb = [np.ascontiguousarray(x[b].T) for b in range(B)]  # [768, 4096]
    kscale = np.float32(SCALE / 16.0)
    for c in range(N_CORES):
        b = c // 4
        hs = HPC * (c % 4)

        def sect(kind, h):  # q=0,k=1,v=2
            lo = kind * (H * DH) + h * DH
            return w_qkv[:, lo : lo + DH], b_qkv[lo : lo + DH]

        q0, bq0 = sect(0, hs); q1, bq1 = sect(0, hs + 1); q2, bq2 = sect(0, hs + 2)
        k0, bk0 = sect(1, hs); k1, bk1 = sect(1, hs + 1); k2, bk2 = sect(1, hs + 2)
        v0, bv0 = sect(2, hs); v1, bv1 = sect(2, hs + 1); v2, bv2 = sect(2, hs + 2)
        z = np.zeros_like(q2); bz = np.zeros_like(bq2)
        # m-chunks: [q0|q1], [q2|q2], [k0|k1]*s, [k2|k2]*s, [v0|v1], [v2|0]
        cols = np.concatenate(
            [q0, q1, q2, q2, k0 * kscale, k1 * kscale, k2 * kscale, k2 * kscale,
             v0, v1, v2, z], axis=1).astype(np.float32)
        bias = np.concatenate(
            [bq0, bq1, bq2, bq2, bk0 * kscale, bk1 * kscale, bk2 * kscale,
             bk2 * kscale, bv0, bv1, bv2, bz]).astype(np.float32)
        q = c % 4
        bo = np.zeros((2, 128), np.float32)
        bo[0, :] = b_out[192 * q : 192 * q + 128]
        bo[1, :64] = b_out[192 * q + 128 : 192 * q + 192]
        in_maps.append({
            "xT": xTb[b],
            "wqkv": np.ascontiguousarray(cols),
            "bqkv": np.ascontiguousarray(bias.reshape(6, 128)),
            "wout": np.ascontiguousarray(
                w_out[:, 192 * q : 192 * (q + 1)].astype(np.float32)),
            "bout": bo,
        })
    return in_maps


def assemble_output(results):
    out = np.empty((B, N, D), dtype=np.float32)
    for c in range(N_CORES):
        b = c // 4
        q = c % 4
        out[b, :, 192 * q : 192 * (q + 1)] = results[c]["y"].T
    return out


def kernel(x, w_qkv, b_qkv, w_out, b_out):
    from concourse.bass_utils import run_bass_kernel_spmd

    x = np.asarray(x, dtype=np.float32)
    nc = build_program()
    in_maps = make_in_maps(
        x, np.asarray(w_qkv, np.float32), np.asarray(b_qkv, np.float32),
        np.asarray(w_out, np.float32), np.asarray(b_out, np.float32))
    res = run_bass_kernel_spmd(nc, in_maps, core_ids=list(range(N_CORES)))
    return assemble_output(res.results)



# revision 23
# speedup vs baseline: 1.3592x; 1.0085x over previous
"""Multi-head self-attention (B=2, N=4096, D=768, H=12, dh=64) on 8 trn2 NeuronCores.

Sharding: core c handles batch b=c//4 and heads 3*(c%4)..3*(c%4)+2 (head-parallel
attention), then an AllToAll redistributes head-outputs so each core projects its
own token quarter with the full w_out (token-parallel output projection).

Per-core pipeline (all matmuls fp32r):
  1. qkv projection from host-transposed xT, producing qT/kT (d-on-partition,
     head-pair packed for PE row-tiling) and v (token-on-partition, via PE
     transpose of vT), with k pre-scaled by softmax_scale/8.
  2. flash-style attention per 512-query chunk: row-tiled QK^T -> exp (split
     between ACT table exp and a custom 1-op DVE polynomial exp16) -> PV
     accumulation with an appended ones-column producing the softmax
     denominator -> normalize (fast DVE reciprocal + K=1 matmul broadcast
     into po rows 64:128).
  3. AllToAll over each batch's 4 cores + output projection + bias.
"""
import sys

sys.path.insert(0, "/opt/trn_rl_repo")

import numpy as np

import concourse.bass as bass
import concourse.mybir as mybir
import concourse.tile as tile
import concourse.bacc as bacc
from concourse.masks import make_identity

N_CORES = 8
B, N, D, H, DH = 2, 4096, 768, 12, 64
HPC = 3            # heads per core
NQ = N // 4        # tokens per core quarter (1024)
SCALE = D ** -0.5
F32 = mybir.dt.float32
F32R = mybir.dt.float32r
AF = mybir.ActivationFunctionType
BF16 = mybir.dt.bfloat16
IC = 512           # query chunk (i-chunk) size
NIC = N // IC      # 8 i-chunks
NTAU = N // IC     # token chunks for projection (512 wide)
VW = 195           # v_sb row stride: [v0|1|v1|1|v2|1] = 3*65
# 5 of every 12 exp batches go to the DVE 1-op exp16 (spread interleave)
DVE_EXP_MOD = 12
DVE_EXP_SLOTS = frozenset((0, 2, 5, 7, 10))


# ---------------------------------------------------------------- custom DVE exp
def _register_exp_ops():
    """exp(16u) in ONE DVE op: (((u+1)^2 + 1) * 0.5)^16 = taylor2(u)^16.

    taylor2(u) = 1 + u + u^2/2 via ((u+1)^2 + 1)/2; 8 ALU stages exactly:
    add, sq, add, mul, sq, sq, sq, sq.  rel err ~ logit^3/1536 (<6e-3 at
    |logit|<=2), negligible after softmax averaging.
    """
    import concourse.dve_ops as dve_ops
    from concourse.dve_ops import DveOp, OPS, CUSTOM_DVE_SPECS, _SUB_OPCODE_FOR_NAME
    from concourse.dve_spec import Spec, Src0, C0, One, sq, lower
    from concourse.dve_uop import DveOpSpec

    if "EXP16_ANT" in _SUB_OPCODE_FOR_NAME:
        return dve_ops.EXP16_ANT

    u = Src0
    spec_e16 = Spec(
        body=sq(sq(sq(sq((sq(u + One) + One) * C0)))),
        reference=lambda in0, in1, s0, s1, imm2: (
            (((in0 + 1.0) ** 2 + 1.0) * s0) ** 16
        ),
    )

    def _mk(name, spec):
        opcode = max(_SUB_OPCODE_FOR_NAME.values()) + 1
        _SUB_OPCODE_FOR_NAME[name] = opcode
        shas = {}
        for ver in ("v3", "v4"):
            s = DveOpSpec(
                name=name, opcode=opcode, uops=lower(spec, ver=ver), rd1_en=False
            )
            shas[ver] = s.sha(ver)
        op = DveOp(name, spec, subdim=False, uops_sha=shas)
        OPS.append(op)
        CUSTOM_DVE_SPECS[name] = spec
        setattr(dve_ops, name, op)
        return op

    return _mk("EXP16_ANT", spec_e16)


# ---------------------------------------------------------------- program build
_PROG_CACHE = {}


def build_program(use_dve_exp=True, use_fast_recip=False):
    key = ("prog", use_dve_exp, use_fast_recip)
    if key in _PROG_CACHE:
        return _PROG_CACHE[key]
    e16_op = _register_exp_ops()

    nc = bacc.Bacc("TRN2", target_bir_lowering=False, debug=False, num_devices=N_CORES)

    xT = nc.dram_tensor("xT", [D, N], BF16, kind="ExternalInput").ap()
    wqkv = nc.dram_tensor("wqkv", [D, 768], BF16, kind="ExternalInput").ap()
    bqkv = nc.dram_tensor("bqkv", [6, 128], F32, kind="ExternalInput").ap()
    wout = nc.dram_tensor("wout", [D, 192], F32, kind="ExternalInput").ap()
    bout = nc.dram_tensor("bout", [2, 128], F32, kind="ExternalInput").ap()
    y = nc.dram_tensor("y", [HPC * DH, N], F32, kind="ExternalOutput").ap()


    with tile.TileContext(nc, trace_sim=False) as tc:
        with (
            tc.tile_pool(name="consts", bufs=1) as consts,
            tc.tile_pool(name="persist", bufs=1) as persist,
            tc.tile_pool(name="otp", bufs=3) as otp,
            tc.tile_pool(name="spsum", bufs=2, space="PSUM") as spsum,
            tc.tile_pool(name="opsum", bufs=1, space="PSUM") as opsum,
            tc.tile_pool(name="dram", bufs=1, space="DRAM") as dram,
            tc.tile_pool(name="ph3", bufs=1) as ph3,
        ):
            # ---------------- constants
            ident_f = consts.tile([128, 128], F32)
            make_identity(nc, ident_f[:])
            ident = consts.tile([128, 128], BF16)
            nc.scalar.copy(ident[:], ident_f[:])

            ones_f = consts.tile([128, 64], F32)
            nc.vector.memset(ones_f[:], 1.0)
            # K=1 stationary for the 1/l broadcast; row 64 so its base
            # partition matches rr's (which must match po's l row for the
            # custom-DVE reciprocal - the DVE lane is tied to the partition).
            ones1_t = consts.tile([128, 64], F32R)
            nc.scalar.copy(ones1_t[:], ones_f[:])
            ones1 = ones1_t[64:65, :]

            wq_sb = consts.tile([128, 6 * 768], BF16)  # 6 f-chunks of [128,768]
            for fc in range(6):
                nc.sync.dma_start(
                    out=wq_sb[:, fc * 768 : (fc + 1) * 768],
                    in_=wqkv[fc * 128 : (fc + 1) * 128, :],
                )
            bq_sb = consts.tile([128, 6], F32)  # per-m-chunk bias columns
            for m in range(6):
                nc.sync.dma_start(
                    out=bq_sb[:, m : m + 1],
                    in_=bqkv[m : m + 1, :].rearrange("a p -> p a"),
                )

            # phase-3 weights, prefetched at program start on the gpsimd queue
            wo_sb = ph3.tile([128, 6 * 192], BF16)  # w_out col-slice, 6 d-chunks
            for dc in range(6):
                nc.gpsimd.dma_start(
                    out=wo_sb[:, dc * 192 : (dc + 1) * 192],
                    in_=wout[dc * 128 : (dc + 1) * 128, :],
                )
            bo_sb = ph3.tile([128, 2], F32)
            for m in range(2):
                nc.sync.dma_start(
                    out=bo_sb[:, m : m + 1],
                    in_=bout[m : m + 1, :].rearrange("a p -> p a"),
                )

            # ---------------- persistent activations
            qT01 = persist.tile([128, N], BF16)  # rows 0:64 head0 qT, 64:128 head1
            qT2 = persist.tile([128, N], BF16)   # head2 qT duplicated in both halves
            kT01 = persist.tile([128, N], BF16)  # scaled kT, heads 0/1
            kT2 = persist.tile([128, N], BF16)   # scaled kT head2, duplicated
            v_sb = persist.tile([128, 32 * VW], BF16)  # [tok128][v0|1|v1|1|v2|1]/j-tile

            # ones columns of v_sb (positions 64,129,194 of each 195 block)
            ones32 = consts.tile([128, 32], F32)
            nc.vector.memset(ones32[:], 1.0)
            v_view = v_sb[:].rearrange("p (t c) -> p t c", c=VW)
            for col in (64, 129, 194):
                nc.scalar.copy(
                    v_view[:, :, col : col + 1],
                    ones32[:].rearrange("p (a b) -> p a b", b=1),
                )

            # AllGather eighths: in [192, 512] -> out [768, 512] (rank-major rows)
            oT_q = [dram.tile([HPC * DH, IC], BF16, name=f"oT_q{i}") for i in range(8)]
            ag_q = [dram.tile([D, IC], BF16, name=f"ag_q{i}") for i in range(8)]

            with tc.tile_pool(name="work", bufs=2) as work, \
                 tc.tile_pool(name="ptp", bufs=3) as ptp:
                # ---------------- phase 1: qkv projection
                qdst = [qT01, qT2, kT01, kT2]
                pending_vt = [None]
                for tau in range(NTAU):
                    ts = slice(tau * IC, (tau + 1) * IC)
                    xts = []
                    for fc in range(6):
                        xt = work.tile([128, IC], BF16, tag="xt", bufs=7)
                        nc.sync.dma_start(
                            out=xt[:], in_=xT[fc * 128 : (fc + 1) * 128, ts]
                        )
                        xts.append(xt)
                    vt01 = work.tile([128, IC], BF16, tag="vt01")
                    vt2 = work.tile([128, IC], BF16, tag="vt2")
                    for m in range(6):
                        pp = spsum.tile([128, 1024], F32, tag="sp", bufs=3)
                        for fc in range(6):
                            nc.tensor.matmul(
                                pp[:, 0:IC],
                                wq_sb[:, fc * 768 + m * 128 : fc * 768 + (m + 1) * 128],
                                xts[fc][:],
                                start=(fc == 0),
                                stop=(fc == 5),
                            )
                        bias = bq_sb[:, m : m + 1]
                        if m < 4:
                            nc.scalar.activation(
                                qdst[m][:, ts], pp[:, 0:IC], AF.Identity, bias=bias
                            )
                        elif m == 4:
                            nc.scalar.activation(
                                vt01[:], pp[:, 0:IC], AF.Identity, bias=bias
                            )
                        else:
                            nc.scalar.activation(
                                vt2[:], pp[:, 0:IC], AF.Identity, bias=bias
                            )
                    # transpose v into token-major layout; deferred one tau
                    # so the transposes don't stall the PE FIFO waiting on
                    # the ACT evacuation of vt01/vt2
                    def vtrans(tau=tau, vt01=vt01, vt2=vt2):
                        for t in range(4):
                            jt = 4 * tau + t
                            base = jt * VW
                            pv = spsum.tile([128, 1024], BF16, tag="sp", bufs=3)
                            nc.tensor.transpose(
                                pv[:, 0:128], vt01[:, t * 128 : (t + 1) * 128],
                                ident[:]
                            )
                            nc.vector.tensor_copy(
                                v_sb[:, base : base + 64], pv[:, 0:64]
                            )
                            nc.vector.tensor_copy(
                                v_sb[:, base + 65 : base + 129], pv[:, 64:128]
                            )
                            pv2 = spsum.tile([128, 1024], BF16, tag="sp", bufs=3)
                            nc.tensor.transpose(
                                pv2[:, 0:64],
                                vt2[0:64, t * 128 : (t + 1) * 128],
                                ident[0:64, 0:64],
                            )
                            nc.vector.tensor_copy(
                                v_sb[:, base + 130 : base + 194], pv2[:, 0:64]
                            )
                    if pending_vt[0] is not None:
                        pending_vt[0]()
                    pending_vt[0] = vtrans
                if pending_vt[0] is not None:
                    pending_vt[0]()
                    pending_vt[0] = None

                # ---------------- phase 2: attention
                exp_batch_idx = [0]

                def exp_batch(sp):
                    """exp(16u) on a [128,1024] psum batch -> fp32r SBUF tile."""
                    i = exp_batch_idx[0]
                    exp_batch_idx[0] += 1
                    pt = ptp.tile([128, 1024], BF16, tag="pt", bufs=5)
                    if use_dve_exp and (i % DVE_EXP_MOD) in DVE_EXP_SLOTS:
                        nc.vector._custom_dve(e16_op, out=pt[:], in0=sp[:], s0=0.5)
                    else:
                        nc.scalar.activation(pt[:], sp[:], AF.Exp, scale=16.0)
                    return pt

                def norm_and_store(po, ic, h):
                    """normalize [65,512] psum (row 64 = l) and DMA to oT_dram.

                    1/l via the fast 1-op DVE reciprocal; broadcast it with a
                    K=1 matmul into the free rows 64:128 of po's own bank
                    (overwriting l, which is dead after the reciprocal), then
                    one DVE multiply.  No ACT work, no spare PSUM tile.
                    """
                    rr_t = otp.tile([128, IC], F32R, tag="rr")
                    rr = rr_t[64:65, :]  # base partition 64 = po's l row
                    if use_fast_recip:
                        from concourse.dve_ops import (
                            RECIP_APPROX_FAST_CONSTS as _RC,
                            RECIPROCAL_APPROX_FAST as _RF,
                        )
                        with nc.allow_low_precision(reason="fp32r rounding of 1/l"):
                            nc.vector._custom_dve(
                                _RF, out=rr, in0=po[64:65, :],
                                s0=_RC["s0"], s1=_RC["s1"], imm2=_RC["imm2"],
                            )
                    else:
                        with nc.allow_low_precision(reason="fp32r is bit-identical fp32"):
                            nc.vector.reciprocal(rr, po[64:65, :])
                    pr = spsum.tile([128, 1024], F32, tag="sp", bufs=3)
                    nc.tensor.matmul(
                        pr[0:64, 0:IC], ones1, rr,
                        start=True, stop=True,
                    )
                    rb = otp.tile([64, IC], F32, tag="rb")
                    nc.scalar.copy(rb[:], pr[0:64, 0:IC])
                    ot = otp.tile([64, IC], BF16, tag="ot")
                    nc.vector.tensor_tensor(
                        out=ot[:], in0=po[0:64, :], in1=rb[:],
                        op=mybir.AluOpType.mult,
                    )
                    nc.sync.dma_start(
                        out=oT_q[ic][64 * h : 64 * h + 64, :], in_=ot[:]
                    )

                # Software-pipelined slot stream: QK(s)+exp(s) are emitted at
                # slot s, but the PV matmuls for slot s are deferred to slot
                # s+2.  The PE engine queue is strict FIFO, so in the naive
                # order every PV stalls the queue waiting on its exp and the
                # PE idles in small gaps each slot (HAM then never reaches
                # K=8/8 and all matmuls run at half clock).  With the 2-slot
                # deferral each exp has ~2 full slot periods to finish before
                # its PV reaches the head of the PE queue.
                pending = []  # deferred PV-emission closures (depth 2)

                def push_slot(pv_fn):
                    pending.append(pv_fn)
                    if len(pending) > 2:
                        pending.pop(0)()

                for ic in range(NIC):
                    isl = slice(ic * IC, (ic + 1) * IC)
                    # --- heads 0,1 (row-tiled pair over the same key tile)
                    po0 = opsum.tile([128, IC], F32, tag="po0", bufs=1)
                    po1 = opsum.tile([128, IC], F32, tag="po1", bufs=1)
                    po2_box = [None]
                    for jc in range(32):
                        sp = spsum.tile([128, 1024], F32, tag="sp", bufs=3)
                        nc.tensor.matmul(
                            sp[:, 0:512],
                            kT01[0:64, jc * 128 : (jc + 1) * 128],
                            qT01[0:64, isl],
                            start=True, stop=True, tile_position=(0, 0),
                        )
                        nc.tensor.matmul(
                            sp[:, 512:1024],
                            kT01[64:128, jc * 128 : (jc + 1) * 128],
                            qT01[64:128, isl],
                            start=True, stop=True, tile_position=(64, 0),
                        )
                        pt = exp_batch(sp)

                        def pv_h01(pt=pt, jc=jc, po0=po0, po1=po1, ic=ic):
                            vbase = jc * VW
                            nc.tensor.matmul(
                                po0[0:65, :], v_sb[:, vbase : vbase + 65],
                                pt[:, 0:512],
                                start=(jc == 0), stop=(jc == 31),
                            )
                            nc.tensor.matmul(
                                po1[0:65, :], v_sb[:, vbase + 65 : vbase + 130],
                                pt[:, 512:1024],
                                start=(jc == 0), stop=(jc == 31),
                            )
                            if jc == 31:
                                norm_and_store(po0, ic, 0)
                                norm_and_store(po1, ic, 1)

                        push_slot(pv_h01)
                    # --- head 2 (row-tiled pair over adjacent key tiles)
                    for t in range(16):
                        sp = spsum.tile([128, 1024], F32, tag="sp", bufs=3)
                        nc.tensor.matmul(
                            sp[:, 0:512],
                            kT2[0:64, (2 * t) * 128 : (2 * t + 1) * 128],
                            qT2[0:64, isl],
                            start=True, stop=True, tile_position=(0, 0),
                        )
                        nc.tensor.matmul(
                            sp[:, 512:1024],
                            kT2[64:128, (2 * t + 1) * 128 : (2 * t + 2) * 128],
                            qT2[64:128, isl],
                            start=True, stop=True, tile_position=(64, 0),
                        )
                        pt = exp_batch(sp)

                        def pv_h2(pt=pt, t=t, ic=ic, po2_box=po2_box):
                            # allocated lazily so the buffer handoff from po0
                            # (shared tag) happens after po0's norm is emitted
                            if po2_box[0] is None:
                                po2_box[0] = opsum.tile(
                                    [128, IC], F32, tag="po0", bufs=1,
                                    name=f"po2_{ic}",
                                )
                            po2 = po2_box[0]
                            for s in range(2):
                                jc = 2 * t + s
                                vbase = jc * VW
                                nc.tensor.matmul(
                                    po2[0:65, :],
                                    v_sb[:, vbase + 130 : vbase + 195],
                                    pt[:, s * 512 : (s + 1) * 512],
                                    start=(jc == 0), stop=(jc == 31),
                                )
                            if t == 15:
                                norm_and_store(po2, ic, 2)
                                nc.gpsimd.collective_compute(
                                    "AllGather",
                                    mybir.AluOpType.bypass,
                                    replica_groups=[[0, 1, 2, 3], [4, 5, 6, 7]],
                                    ins=[oT_q[ic][:]],
                                    outs=[ag_q[ic][:]],
                                )

                        push_slot(pv_h2)
                while pending:
                    pending.pop(0)()

            # ---------------- phase 3: output projection (column-parallel)
            for qtr in range(8):
                agr = ag_q[qtr][:]
                ogs = []
                for dc in range(6):
                    og = ph3.tile([128, IC], BF16, tag="og", bufs=8)
                    nc.gpsimd.dma_start(
                        out=og[:], in_=agr[dc * 128 : (dc + 1) * 128, :]
                    )
                    ogs.append(og)
                for ec, (elo, ew) in enumerate(((0, 128), (128, 64))):
                    py = spsum.tile([128, 1024], F32, tag="sp", bufs=3)
                    for dc in range(6):
                        nc.tensor.matmul(
                            py[0:ew, 0:IC],
                            wo_sb[:, dc * 192 + elo : dc * 192 + elo + ew],
                            ogs[dc][:],
                            start=(dc == 0), stop=(dc == 5),
                        )
                    ysb = ph3.tile([128, IC], F32, tag="ysb", bufs=3)
                    nc.scalar.activation(
                        ysb[0:ew, :], py[0:ew, 0:IC], AF.Identity,
                        bias=bo_sb[0:ew, ec : ec + 1],
                    )
                    nc.sync.dma_start(
                        out=y[elo : elo + ew,
                              qtr * IC : (qtr + 1) * IC],
                        in_=ysb[0:ew, :],
                    )

    nc.compile()
    _PROG_CACHE[key] = nc
    return nc


# ---------------------------------------------------------------- host wrapper
def make_in_maps(x, w_qkv, b_qkv, w_out, b_out):
    """Build the 8 per-core input dicts from full inputs."""
    in_maps = []
    import ml_dtypes
    bf16 = ml_dtypes.bfloat16
    xTb = [np.ascontiguousarray(x[b].T.astype(bf16)) for b in range(B)]  # [768, 4096]
    kscale = np.float32(SCALE / 16.0)
    for c in range(N_CORES):
        b = c // 4
        hs = HPC * (c % 4)

        def sect(kind, h):  # q=0,k=1,v=2
            lo = kind * (H * DH) + h * DH
            return w_qkv[:, lo : lo + DH], b_qkv[lo : lo + DH]

        q0, bq0 = sect(0, hs); q1, bq1 = sect(0, hs + 1); q2, bq2 = sect(0, hs + 2)
        k0, bk0 = sect(1, hs); k1, bk1 = sect(1, hs + 1); k2, bk2 = sect(1, hs + 2)
        v0, bv0 = sect(2, hs); v1, bv1 = sect(2, hs + 1); v2, bv2 = sect(2, hs + 2)
        z = np.zeros_like(q2); bz = np.zeros_like(bq2)
        # m-chunks: [q0|q1], [q2|q2], [k0|k1]*s, [k2|k2]*s, [v0|v1], [v2|0]
        cols = np.concatenate(
            [q0, q1, q2, q2, k0 * kscale, k1 * kscale, k2 * kscale, k2 * kscale,
             v0, v1, v2, z], axis=1).astype(np.float32)
        bias = np.concatenate(
            [bq0, bq1, bq2, bq2, bk0 * kscale, bk1 * kscale, bk2 * kscale,
             bk2 * kscale, bv0, bv1, bv2, bz]).astype(np.float32)
        q = c % 4
        bo = np.zeros((2, 128), np.float32)
        bo[0, :] = b_out[192 * q : 192 * q + 128]
        bo[1, :64] = b_out[192 * q + 128 : 192 * q + 192]
        in_maps.append({
            "xT": xTb[b],
            "wqkv": np.ascontiguousarray(cols.astype(bf16)),
            "bqkv": np.ascontiguousarray(bias.reshape(6, 128)),
            "wout": np.ascontiguousarray(
                w_out[:, 192 * q : 192 * (q + 1)].astype(np.float32)),
            "bout": bo,
        })
    return in_maps


def assemble_output(results):
    out = np.empty((B, N, D), dtype=np.float32)
    for c in range(N_CORES):
        b = c // 4
        q = c % 4
        out[b, :, 192 * q : 192 * (q + 1)] = results[c]["y"].T
    return out


def kernel(x, w_qkv, b_qkv, w_out, b_out):
    from concourse.bass_utils import run_bass_kernel_spmd

    x = np.asarray(x, dtype=np.float32)
    nc = build_program()
    in_maps = make_in_maps(
        x, np.asarray(w_qkv, np.float32), np.asarray(b_qkv, np.float32),
        np.asarray(w_out, np.float32), np.asarray(b_out, np.float32))
    res = run_bass_kernel_spmd(nc, in_maps, core_ids=list(range(N_CORES)))
    return assemble_output(res.results)



# revision 25
# speedup vs baseline: 1.4895x; 1.0959x over previous
"""Multi-head self-attention (B=2, N=4096, D=768, H=12, dh=64) on 8 trn2 NeuronCores.

Sharding: core c handles batch b=c//4 and heads 3*(c%4)..3*(c%4)+2 (head-parallel
attention), then an AllToAll redistributes head-outputs so each core projects its
own token quarter with the full w_out (token-parallel output projection).

Per-core pipeline (all matmuls fp32r):
  1. qkv projection from host-transposed xT, producing qT/kT (d-on-partition,
     head-pair packed for PE row-tiling) and v (token-on-partition, via PE
     transpose of vT), with k pre-scaled by softmax_scale/8.
  2. flash-style attention per 512-query chunk: row-tiled QK^T -> exp (split
     between ACT table exp and a custom 1-op DVE polynomial exp16) -> PV
     accumulation with an appended ones-column producing the softmax
     denominator -> normalize (fast DVE reciprocal + K=1 matmul broadcast
     into po rows 64:128).
  3. AllToAll over each batch's 4 cores + output projection + bias.
"""
import sys

sys.path.insert(0, "/opt/trn_rl_repo")

import numpy as np

import concourse.bass as bass
import concourse.mybir as mybir
import concourse.tile as tile
import concourse.bacc as bacc
from concourse.masks import make_identity

N_CORES = 8
B, N, D, H, DH = 2, 4096, 768, 12, 64
HPC = 3            # heads per core
NQ = N // 4        # tokens per core quarter (1024)
SCALE = D ** -0.5
F32 = mybir.dt.float32
F32R = mybir.dt.float32r
AF = mybir.ActivationFunctionType
BF16 = mybir.dt.bfloat16
IC = 512           # query chunk (i-chunk) size
NIC = N // IC      # 8 i-chunks
NTAU = N // IC     # token chunks for projection (512 wide)
VW = 195           # v_sb row stride: [v0|1|v1|1|v2|1] = 3*65
# 5 of every 12 exp batches go to the DVE 1-op exp16 (spread interleave)
DVE_EXP_MOD = 12
DVE_EXP_SLOTS = frozenset((0, 2, 5, 7, 10))


# ---------------------------------------------------------------- custom DVE exp
def _register_exp_ops():
    """exp(16u) in ONE DVE op: (((u+1)^2 + 1) * 0.5)^16 = taylor2(u)^16.

    taylor2(u) = 1 + u + u^2/2 via ((u+1)^2 + 1)/2; 8 ALU stages exactly:
    add, sq, add, mul, sq, sq, sq, sq.  rel err ~ logit^3/1536 (<6e-3 at
    |logit|<=2), negligible after softmax averaging.
    """
    import concourse.dve_ops as dve_ops
    from concourse.dve_ops import DveOp, OPS, CUSTOM_DVE_SPECS, _SUB_OPCODE_FOR_NAME
    from concourse.dve_spec import Spec, Src0, C0, One, sq, lower
    from concourse.dve_uop import DveOpSpec

    if "EXP16_ANT" in _SUB_OPCODE_FOR_NAME:
        return dve_ops.EXP16_ANT

    u = Src0
    spec_e16 = Spec(
        body=sq(sq(sq(sq((sq(u + One) + One) * C0)))),
        reference=lambda in0, in1, s0, s1, imm2: (
            (((in0 + 1.0) ** 2 + 1.0) * s0) ** 16
        ),
    )

    def _mk(name, spec):
        opcode = max(_SUB_OPCODE_FOR_NAME.values()) + 1
        _SUB_OPCODE_FOR_NAME[name] = opcode
        shas = {}
        for ver in ("v3", "v4"):
            s = DveOpSpec(
                name=name, opcode=opcode, uops=lower(spec, ver=ver), rd1_en=False
            )
            shas[ver] = s.sha(ver)
        op = DveOp(name, spec, subdim=False, uops_sha=shas)
        OPS.append(op)
        CUSTOM_DVE_SPECS[name] = spec
        setattr(dve_ops, name, op)
        return op

    return _mk("EXP16_ANT", spec_e16)


# ---------------------------------------------------------------- program build
_PROG_CACHE = {}


def build_program(use_dve_exp=True, use_fast_recip=True):
    key = ("prog", use_dve_exp, use_fast_recip)
    if key in _PROG_CACHE:
        return _PROG_CACHE[key]
    e16_op = _register_exp_ops()

    nc = bacc.Bacc("TRN2", target_bir_lowering=False, debug=False, num_devices=N_CORES)

    xT = nc.dram_tensor("xT", [D, N], BF16, kind="ExternalInput").ap()
    wqkv = nc.dram_tensor("wqkv", [D, 768], BF16, kind="ExternalInput").ap()
    bqkv = nc.dram_tensor("bqkv", [6, 128], F32, kind="ExternalInput").ap()
    wout = nc.dram_tensor("wout", [D, 192], F32, kind="ExternalInput").ap()
    bout = nc.dram_tensor("bout", [2, 128], F32, kind="ExternalInput").ap()
    y = nc.dram_tensor("y", [HPC * DH, N], F32, kind="ExternalOutput").ap()


    with tile.TileContext(nc, trace_sim=False) as tc:
        with (
            tc.tile_pool(name="consts", bufs=1) as consts,
            tc.tile_pool(name="persist", bufs=1) as persist,
            tc.tile_pool(name="otp", bufs=3) as otp,
            tc.tile_pool(name="spsum", bufs=2, space="PSUM") as spsum,
            tc.tile_pool(name="opsum", bufs=1, space="PSUM") as opsum,
            tc.tile_pool(name="dram", bufs=1, space="DRAM") as dram,
            tc.tile_pool(name="ph3", bufs=1) as ph3,
        ):
            # ---------------- constants
            ident_f = consts.tile([128, 128], F32)
            make_identity(nc, ident_f[:])
            ident = consts.tile([128, 128], BF16)
            nc.scalar.copy(ident[:], ident_f[:])

            ones_f = consts.tile([128, 64], F32)
            nc.vector.memset(ones_f[:], 1.0)
            # K=1 stationary for the 1/l broadcast (base partition 0, same
            # as rr and po's l row - custom-DVE lanes are tied to partitions)
            ones1_t = consts.tile([128, 64], F32R)
            nc.scalar.copy(ones1_t[:], ones_f[:])
            ones1 = ones1_t[0:1, :]

            wq_sb = consts.tile([128, 6 * 768], BF16)  # 6 f-chunks of [128,768]
            for fc in range(6):
                nc.sync.dma_start(
                    out=wq_sb[:, fc * 768 : (fc + 1) * 768],
                    in_=wqkv[fc * 128 : (fc + 1) * 128, :],
                )
            bq_sb = consts.tile([128, 6], F32)  # per-m-chunk bias columns
            for m in range(6):
                nc.sync.dma_start(
                    out=bq_sb[:, m : m + 1],
                    in_=bqkv[m : m + 1, :].rearrange("a p -> p a"),
                )

            # phase-3 weights, prefetched at program start on the gpsimd queue
            wo_sb = ph3.tile([128, 6 * 192], BF16)  # w_out col-slice, 6 d-chunks
            for dc in range(6):
                nc.gpsimd.dma_start(
                    out=wo_sb[:, dc * 192 : (dc + 1) * 192],
                    in_=wout[dc * 128 : (dc + 1) * 128, :],
                )
            bo_sb = ph3.tile([128, 2], F32)
            for m in range(2):
                nc.sync.dma_start(
                    out=bo_sb[:, m : m + 1],
                    in_=bout[m : m + 1, :].rearrange("a p -> p a"),
                )

            # ---------------- persistent activations
            qT01 = persist.tile([128, N], BF16)  # rows 0:64 head0 qT, 64:128 head1
            qT2 = persist.tile([128, N], BF16)   # head2 qT duplicated in both halves
            kT01 = persist.tile([128, N], BF16)  # scaled kT, heads 0/1
            kT2 = persist.tile([128, N], BF16)   # scaled kT head2, duplicated
            v_sb = persist.tile([128, 32 * VW], BF16)  # [tok128][v0|1|v1|1|v2|1]/j-tile

            # ones columns of v_sb (positions 64,129,194 of each 195 block)
            ones32 = consts.tile([128, 32], F32)
            nc.vector.memset(ones32[:], 1.0)
            v_view = v_sb[:].rearrange("p (t c) -> p t c", c=VW)
            for col in (64, 129, 194):
                nc.scalar.copy(
                    v_view[:, :, col : col + 1],
                    ones32[:].rearrange("p (a b) -> p a b", b=1),
                )

            # AllGather eighths: in [192, 512] -> out [768, 512] (rank-major rows)
            oT_q = [dram.tile([HPC * DH, IC], BF16, name=f"oT_q{i}") for i in range(8)]
            ag_q = [dram.tile([D, IC], BF16, name=f"ag_q{i}") for i in range(8)]

            with tc.tile_pool(name="work", bufs=2) as work, \
                 tc.tile_pool(name="ptp", bufs=3) as ptp:
                # ---------------- phase 1: qkv projection
                qdst = [qT01, qT2, kT01, kT2]
                pending_vt = [None]
                for tau in range(NTAU):
                    ts = slice(tau * IC, (tau + 1) * IC)
                    xts = []
                    for fc in range(6):
                        xt = work.tile([128, IC], BF16, tag="xt", bufs=7)
                        nc.sync.dma_start(
                            out=xt[:], in_=xT[fc * 128 : (fc + 1) * 128, ts]
                        )
                        xts.append(xt)
                    vt01 = work.tile([128, IC], BF16, tag="vt01")
                    vt2 = work.tile([128, IC], BF16, tag="vt2")
                    for m in range(6):
                        pp = spsum.tile([128, 1024], F32, tag="sp", bufs=3)
                        for fc in range(6):
                            nc.tensor.matmul(
                                pp[:, 0:IC],
                                wq_sb[:, fc * 768 + m * 128 : fc * 768 + (m + 1) * 128],
                                xts[fc][:],
                                start=(fc == 0),
                                stop=(fc == 5),
                            )
                        bias = bq_sb[:, m : m + 1]
                        if m < 4:
                            nc.scalar.activation(
                                qdst[m][:, ts], pp[:, 0:IC], AF.Identity, bias=bias
                            )
                        elif m == 4:
                            nc.scalar.activation(
                                vt01[:], pp[:, 0:IC], AF.Identity, bias=bias
                            )
                        else:
                            nc.scalar.activation(
                                vt2[:], pp[:, 0:IC], AF.Identity, bias=bias
                            )
                    # transpose v into token-major layout; deferred one tau
                    # so the transposes don't stall the PE FIFO waiting on
                    # the ACT evacuation of vt01/vt2
                    def vtrans(tau=tau, vt01=vt01, vt2=vt2):
                        for t in range(4):
                            jt = 4 * tau + t
                            base = jt * VW
                            pv = spsum.tile([128, 1024], BF16, tag="sp", bufs=3)
                            nc.tensor.transpose(
                                pv[:, 0:128], vt01[:, t * 128 : (t + 1) * 128],
                                ident[:]
                            )
                            nc.vector.tensor_copy(
                                v_sb[:, base : base + 64], pv[:, 0:64]
                            )
                            nc.vector.tensor_copy(
                                v_sb[:, base + 65 : base + 129], pv[:, 64:128]
                            )
                            pv2 = spsum.tile([128, 1024], BF16, tag="sp", bufs=3)
                            nc.tensor.transpose(
                                pv2[:, 0:64],
                                vt2[0:64, t * 128 : (t + 1) * 128],
                                ident[0:64, 0:64],
                            )
                            nc.vector.tensor_copy(
                                v_sb[:, base + 130 : base + 194], pv2[:, 0:64]
                            )
                    if pending_vt[0] is not None:
                        pending_vt[0]()
                    pending_vt[0] = vtrans
                if pending_vt[0] is not None:
                    pending_vt[0]()
                    pending_vt[0] = None

                # ---------------- phase 2: attention
                exp_batch_idx = [0]

                def exp_batch(sp):
                    """exp(16u) on a [128,1024] psum batch -> fp32r SBUF tile."""
                    i = exp_batch_idx[0]
                    exp_batch_idx[0] += 1
                    pt = ptp.tile([128, 1024], BF16, tag="pt", bufs=5)
                    if use_dve_exp and (i % DVE_EXP_MOD) in DVE_EXP_SLOTS:
                        nc.vector._custom_dve(e16_op, out=pt[:], in0=sp[:], s0=0.5)
                    else:
                        nc.scalar.activation(pt[:], sp[:], AF.Exp, scale=16.0)
                    return pt

                def norm_and_store(po, ic, h):
                    """normalize [65,512] psum (row 64 = l) and DMA to oT_dram.

                    The custom-DVE fast reciprocal needs base-partition-0 APs,
                    so ACT first stages l from psum partition 64 to an SBUF
                    row at partition 0 (standard ops handle the cross-base
                    move).  Then 1-op reciprocal, K=1 broadcast matmul, ACT
                    evacuation, one DVE multiply.
                    """
                    rr_t = otp.tile([1, IC], F32R, tag="rr")
                    rr = rr_t[:]
                    if use_fast_recip:
                        from concourse.dve_ops import (
                            RECIP_APPROX_FAST_CONSTS as _RC,
                            RECIPROCAL_APPROX_FAST as _RF,
                        )
                        lsb = otp.tile([1, IC], F32, tag="lsb")
                        nc.scalar.copy(lsb[:], po[64:65, :])
                        with nc.allow_low_precision(reason="fp32r rounding of 1/l"):
                            nc.vector._custom_dve(
                                _RF, out=rr, in0=lsb[:],
                                s0=_RC["s0"], s1=_RC["s1"], imm2=_RC["imm2"],
                            )
                    else:
                        with nc.allow_low_precision(reason="fp32r is bit-identical fp32"):
                            nc.vector.reciprocal(rr, po[64:65, :])
                    pr = spsum.tile([128, 1024], F32, tag="sp", bufs=3)
                    nc.tensor.matmul(
                        pr[0:64, 0:IC], ones1, rr,
                        start=True, stop=True,
                    )
                    rb = otp.tile([64, IC], F32, tag="rb")
                    nc.scalar.copy(rb[:], pr[0:64, 0:IC])
                    ot = otp.tile([64, IC], BF16, tag="ot")
                    nc.vector.tensor_tensor(
                        out=ot[:], in0=po[0:64, :], in1=rb[:],
                        op=mybir.AluOpType.mult,
                    )
                    nc.sync.dma_start(
                        out=oT_q[ic][64 * h : 64 * h + 64, :], in_=ot[:]
                    )

                # Software-pipelined slot stream: QK(s)+exp(s) are emitted at
                # slot s, but the PV matmuls for slot s are deferred to slot
                # s+2.  The PE engine queue is strict FIFO, so in the naive
                # order every PV stalls the queue waiting on its exp and the
                # PE idles in small gaps each slot (HAM then never reaches
                # K=8/8 and all matmuls run at half clock).  With the 2-slot
                # deferral each exp has ~2 full slot periods to finish before
                # its PV reaches the head of the PE queue.
                pending = []  # deferred PV-emission closures (depth 2)

                def push_slot(pv_fn):
                    pending.append(pv_fn)
                    if len(pending) > 2:
                        pending.pop(0)()

                for ic in range(NIC):
                    isl = slice(ic * IC, (ic + 1) * IC)
                    # --- heads 0,1 (row-tiled pair over the same key tile)
                    po0 = opsum.tile([128, IC], F32, tag="po0", bufs=1)
                    po1 = opsum.tile([128, IC], F32, tag="po1", bufs=1)
                    po2_box = [None]
                    for jc in range(32):
                        sp = spsum.tile([128, 1024], F32, tag="sp", bufs=3)
                        nc.tensor.matmul(
                            sp[:, 0:512],
                            kT01[0:64, jc * 128 : (jc + 1) * 128],
                            qT01[0:64, isl],
                            start=True, stop=True, tile_position=(0, 0),
                        )
                        nc.tensor.matmul(
                            sp[:, 512:1024],
                            kT01[64:128, jc * 128 : (jc + 1) * 128],
                            qT01[64:128, isl],
                            start=True, stop=True, tile_position=(64, 0),
                        )
                        pt = exp_batch(sp)

                        def pv_h01(pt=pt, jc=jc, po0=po0, po1=po1, ic=ic):
                            vbase = jc * VW
                            nc.tensor.matmul(
                                po0[0:65, :], v_sb[:, vbase : vbase + 65],
                                pt[:, 0:512],
                                start=(jc == 0), stop=(jc == 31),
                            )
                            nc.tensor.matmul(
                                po1[0:65, :], v_sb[:, vbase + 65 : vbase + 130],
                                pt[:, 512:1024],
                                start=(jc == 0), stop=(jc == 31),
                            )
                            if jc == 31:
                                norm_and_store(po0, ic, 0)
                                norm_and_store(po1, ic, 1)

                        push_slot(pv_h01)
                    # --- head 2 (row-tiled pair over adjacent key tiles)
                    for t in range(16):
                        sp = spsum.tile([128, 1024], F32, tag="sp", bufs=3)
                        nc.tensor.matmul(
                            sp[:, 0:512],
                            kT2[0:64, (2 * t) * 128 : (2 * t + 1) * 128],
                            qT2[0:64, isl],
                            start=True, stop=True, tile_position=(0, 0),
                        )
                        nc.tensor.matmul(
                            sp[:, 512:1024],
                            kT2[64:128, (2 * t + 1) * 128 : (2 * t + 2) * 128],
                            qT2[64:128, isl],
                            start=True, stop=True, tile_position=(64, 0),
                        )
                        pt = exp_batch(sp)

                        def pv_h2(pt=pt, t=t, ic=ic, po2_box=po2_box):
                            # allocated lazily so the buffer handoff from po0
                            # (shared tag) happens after po0's norm is emitted
                            if po2_box[0] is None:
                                po2_box[0] = opsum.tile(
                                    [128, IC], F32, tag="po0", bufs=1,
                                    name=f"po2_{ic}",
                                )
                            po2 = po2_box[0]
                            for s in range(2):
                                jc = 2 * t + s
                                vbase = jc * VW
                                nc.tensor.matmul(
                                    po2[0:65, :],
                                    v_sb[:, vbase + 130 : vbase + 195],
                                    pt[:, s * 512 : (s + 1) * 512],
                                    start=(jc == 0), stop=(jc == 31),
                                )
                            if t == 15:
                                norm_and_store(po2, ic, 2)
                                nc.gpsimd.collective_compute(
                                    "AllGather",
                                    mybir.AluOpType.bypass,
                                    replica_groups=[[0, 1, 2, 3], [4, 5, 6, 7]],
                                    ins=[oT_q[ic][:]],
                                    outs=[ag_q[ic][:]],
                                )

                        push_slot(pv_h2)
                while pending:
                    pending.pop(0)()

            # ---------------- phase 3: output projection (column-parallel)
            for qtr in range(8):
                agr = ag_q[qtr][:]
                ogs = []
                for dc in range(6):
                    og = ph3.tile([128, IC], BF16, tag="og", bufs=8)
                    nc.gpsimd.dma_start(
                        out=og[:], in_=agr[dc * 128 : (dc + 1) * 128, :]
                    )
                    ogs.append(og)
                for ec, (elo, ew) in enumerate(((0, 128), (128, 64))):
                    py = spsum.tile([128, 1024], F32, tag="sp", bufs=3)
                    for dc in range(6):
                        nc.tensor.matmul(
                            py[0:ew, 0:IC],
                            wo_sb[:, dc * 192 + elo : dc * 192 + elo + ew],
                            ogs[dc][:],
                            start=(dc == 0), stop=(dc == 5),
                        )
                    ysb = ph3.tile([128, IC], F32, tag="ysb", bufs=3)
                    nc.scalar.activation(
                        ysb[0:ew, :], py[0:ew, 0:IC], AF.Identity,
                        bias=bo_sb[0:ew, ec : ec + 1],
                    )
                    nc.sync.dma_start(
                        out=y[elo : elo + ew,
                              qtr * IC : (qtr + 1) * IC],
                        in_=ysb[0:ew, :],
                    )

    nc.compile()
    _PROG_CACHE[key] = nc
    return nc


# ---------------------------------------------------------------- host wrapper
def make_in_maps(x, w_qkv, b_qkv, w_out, b_out):
    """Build the 8 per-core input dicts from full inputs."""
    in_maps = []
    import ml_dtypes
    bf16 = ml_dtypes.bfloat16
    xTb = [np.ascontiguousarray(x[b].T.astype(bf16)) for b in range(B)]  # [768, 4096]
    kscale = np.float32(SCALE / 16.0)
    for c in range(N_CORES):
        b = c // 4
        hs = HPC * (c % 4)

        def sect(kind, h):  # q=0,k=1,v=2
            lo = kind * (H * DH) + h * DH
            return w_qkv[:, lo : lo + DH], b_qkv[lo : lo + DH]

        q0, bq0 = sect(0, hs); q1, bq1 = sect(0, hs + 1); q2, bq2 = sect(0, hs + 2)
        k0, bk0 = sect(1, hs); k1, bk1 = sect(1, hs + 1); k2, bk2 = sect(1, hs + 2)
        v0, bv0 = sect(2, hs); v1, bv1 = sect(2, hs + 1); v2, bv2 = sect(2, hs + 2)
        z = np.zeros_like(q2); bz = np.zeros_like(bq2)
        # m-chunks: [q0|q1], [q2|q2], [k0|k1]*s, [k2|k2]*s, [v0|v1], [v2|0]
        cols = np.concatenate(
            [q0, q1, q2, q2, k0 * kscale, k1 * kscale, k2 * kscale, k2 * kscale,
             v0, v1, v2, z], axis=1).astype(np.float32)
        bias = np.concatenate(
            [bq0, bq1, bq2, bq2, bk0 * kscale, bk1 * kscale, bk2 * kscale,
             bk2 * kscale, bv0, bv1, bv2, bz]).astype(np.float32)
        q = c % 4
        bo = np.zeros((2, 128), np.float32)
        bo[0, :] = b_out[192 * q : 192 * q + 128]
        bo[1, :64] = b_out[192 * q + 128 : 192 * q + 192]
        in_maps.append({
            "xT": xTb[b],
            "wqkv": np.ascontiguousarray(cols.astype(bf16)),
            "bqkv": np.ascontiguousarray(bias.reshape(6, 128)),
            "wout": np.ascontiguousarray(
                w_out[:, 192 * q : 192 * (q + 1)].astype(np.float32)),
            "bout": bo,
        })
    return in_maps


def assemble_output(results):
    out = np.empty((B, N, D), dtype=np.float32)
    for c in range(N_CORES):
        b = c // 4
        q = c % 4
        out[b, :, 192 * q : 192 * (q + 1)] = results[c]["y"].T
    return out


def kernel(x, w_qkv, b_qkv, w_out, b_out):
    from concourse.bass_utils import run_bass_kernel_spmd

    x = np.asarray(x, dtype=np.float32)
    nc = build_program()
    in_maps = make_in_maps(
        x, np.asarray(w_qkv, np.float32), np.asarray(b_qkv, np.float32),
        np.asarray(w_out, np.float32), np.asarray(b_out, np.float32))
    res = run_bass_kernel_spmd(nc, in_maps, core_ids=list(range(N_CORES)))
    return assemble_output(res.results)

